# revision 1
# baseline (speedup 1.0000x reference)
"""GraphSAGE mean-aggregation layer on 8 Trainium2 NeuronCores (raw Bass).

Math: out = D^{-1} A (x @ W + b)  ==  (D^{-1} A x) @ W + mask (outer) b
where A is the (row=dest, col=src) adjacency from edge_index, D = row degrees,
mask[d] = 1 if deg[d] > 0 else 0 (zero-degree rows are exactly 0 in the ref).

Strategy (one SPMD program on 8 cores, dest nodes sharded):
  - Host: sort edges by dest, bucket into 128-dest windows (wpc per core), pad
    each window to T tiles of 128 edges. Per-edge weight 1/deg[dest] is folded
    into the selection matrix so PSUM accumulation yields D^{-1}Ax directly.
  - Device, per window: one indirect-DMA gather of T*128 source rows (one row
    per partition per tile), then per 128-edge tile a DVE-built weighted
    one-hot S (S[e,j] = (dst_local[e]==j)*w[e]) and a PE matmul S^T @ G
    accumulating into PSUM [128 dests, 256]; transpose + W matmul + masked
    bias (K=1 outer product), DMA 128 output rows out.
  - Raw bass engine programs with explicit semaphores: this toolchain allows
    only ONE sync wait per instruction, so all waits are standalone wait_ge.
"""

import numpy as np

import concourse.bass as bass
import concourse.mybir as mybir
from concourse.bass_utils import run_bass_kernel_spmd

P = 128
F = 256

N_NODES = 100000
N_CORES = 8
NPC = N_NODES // N_CORES  # dest rows per core


def build_nc(n_nodes, npc, n_tiles, x_dtype=mybir.dt.float32, repeat=1):
    """One SPMD Bass program; n_tiles = edge tiles per 128-dest window."""
    wpc = (npc + P - 1) // P
    T = n_tiles
    f = F
    kf = f // P  # 2 feature chunks of 128
    NG = 2  # gather buffers
    dt_f32 = mybir.dt.float32

    nc = bass.Bass()

    x_h = nc.declare_dram_parameter("x", [n_nodes, f], x_dtype, isOutput=False)
    idx_h = nc.declare_dram_parameter("srcidx", [P, wpc * T], mybir.dt.int32, isOutput=False)
    dw_h = nc.declare_dram_parameter("dw", [P, wpc * 2 * T], dt_f32, isOutput=False)
    msk_h = nc.declare_dram_parameter("maskw", [wpc, P], dt_f32, isOutput=False)
    w_h = nc.declare_dram_parameter("Wm", [f, f], dt_f32, isOutput=False)
    b_h = nc.declare_dram_parameter("bv", [1, f], dt_f32, isOutput=False)
    out_h = nc.declare_dram_parameter("out", [npc, f], dt_f32, isOutput=True)

    NS = T + 12  # S-tile ring: one window + pipeline margin

    from contextlib import ExitStack

    ctx = ExitStack()
    with ctx:
        sb = lambda name, shape, dt: ctx.enter_context(nc.sbuf_tensor(name, shape, dt))
        ps = lambda name, shape: ctx.enter_context(nc.psum_tensor(name, shape, dt_f32))
        sem = lambda name: ctx.enter_context(nc.semaphore(name))

        iota_f = sb("iota_f", [P, P], dt_f32)
        ident = sb("ident", [P, P], dt_f32)
        w0 = sb("w0", [P, f], dt_f32)
        w1 = sb("w1", [P, f], dt_f32)
        b_sb = sb("b_sb", [1, f], dt_f32)
        idx_all = sb("idx_all", [P, wpc * T], mybir.dt.int32)
        dw_all = sb("dw_all", [P, wpc * 2 * T], dt_f32)
        msk_t = sb("msk_t", [1, 2 * P], dt_f32)
        g_buf = sb("g_buf", [P, NG * T * f], x_dtype)
        s_buf = sb("s_buf", [P, NS * P], x_dtype)
        agg_sb = sb("agg_sb", [P, 2 * f], dt_f32)
        tp_sb = sb("tp_sb", [P, kf * P], dt_f32)
        out_sb = sb("out_sb", [P, 2 * f], dt_f32)
        agg_ps = [ps("agg_ps0", [P, f]), ps("agg_ps1", [P, f])]
        tp_ps = [ps("tp_ps0", [P, P]), ps("tp_ps1", [P, P])]
        out_ps = [ps("out_ps0", [P, f]), ps("out_ps1", [P, f])]
        SEM_META = sem("sem_meta")
        SEM_CONST = sem("sem_const")
        SEM_G = sem("sem_g")
        SEM_S = sem("sem_s")
        SEM_MM = sem("sem_mm")
        SEM_CP = sem("sem_cp")
        SEM_TP = sem("sem_tp")
        SEM_TPC = sem("sem_tpc")
        SEM_FIN = sem("sem_fin")
        SEM_OUT = sem("sem_out")
        SEM_OD = sem("sem_od")
        SEM_MSK = sem("sem_msk")

        w_sb = [w0, w1]

        with nc.Block() as block:

            @block.sync
            def _(sync):
                # startup loads (HWDGE)
                sync.dma_start(w0[:, :], w_h[0:P, :]).then_inc(SEM_META, 16)
                sync.dma_start(w1[:, :], w_h[P : 2 * P, :]).then_inc(SEM_META, 16)
                sync.dma_start(b_sb[:, :], b_h[:, :]).then_inc(SEM_META, 16)
                sync.dma_start(idx_all[:, :], idx_h[:, :]).then_inc(SEM_META, 16)
                sync.dma_start(dw_all[:, :], dw_h[:, :]).then_inc(SEM_META, 16)
                # per-window mask loads + output stores
                for W in range(repeat * wpc):
                    w = W % wpc
                    rows = min(P, npc - w * P)
                    ob = (W % 2) * f
                    mb = (W % 2) * P
                    if W >= 2:
                        sync.wait_ge(SEM_FIN, W - 1)  # msk_t slot free
                    sync.dma_start(
                        msk_t[:1, mb : mb + P], msk_h[w : w + 1, :]
                    ).then_inc(SEM_MSK, 16)
                    sync.wait_ge(SEM_OUT, W + 1)
                    sync.dma_start(
                        out_h[w * P : w * P + rows, :], out_sb[:rows, ob : ob + f]
                    ).then_inc(SEM_OD, 16)

            @block.gpsimd
            def _(gpsimd):
                # constants
                gpsimd.iota(
                    iota_f[:, :],
                    pattern=[[1, P]],
                    base=0,
                    channel_multiplier=0,
                    allow_small_or_imprecise_dtypes=True,
                )
                gpsimd.memset(ident[:, :], 0.0)
                gpsimd.affine_select(
                    out=ident[:, :],
                    in_=ident[:, :],
                    compare_op=mybir.AluOpType.not_equal,
                    fill=1.0,
                    base=0,
                    pattern=[[-1, P]],
                    channel_multiplier=1,
                ).then_inc(SEM_CONST, 1)
                # gathers
                gpsimd.wait_ge(SEM_META, 80)
                for W in range(repeat * wpc):
                    w = W % wpc
                    gb = (W % NG) * T * f
                    if W >= NG:
                        # g buffer free once PE finished window W-NG's matmuls
                        gpsimd.wait_ge(SEM_MM, (W - NG + 1) * T)
                    for t in range(T):
                        # HW indirect DMA honors ONE offset per partition:
                        # one call per 128-edge tile.
                        gpsimd.indirect_dma_start(
                            out=g_buf[:, gb + t * f : gb + (t + 1) * f],
                            out_offset=None,
                            in_=x_h[:, :],
                            in_offset=bass.IndirectOffsetOnAxis(
                                ap=idx_all[:, w * T + t : w * T + t + 1], axis=0
                            ),
                        ).then_inc(SEM_G, 16)

            @block.vector
            def _(vector):
                vector.wait_ge(SEM_CONST, 1)
                vector.wait_ge(SEM_META, 80)
                for W in range(repeat * wpc):
                    w = W % wpc
                    # build S tiles for window w
                    for t in range(T):
                        i = W * T + t
                        sb = (i % NS) * P
                        if i >= NS:
                            vector.wait_ge(SEM_MM, i - NS + 1)
                        vector.tensor_scalar(
                            out=s_buf[:, sb : sb + P],
                            in0=iota_f[:, :],
                            scalar1=dw_all[:, w * 2 * T + t : w * 2 * T + t + 1],
                            scalar2=dw_all[:, w * 2 * T + T + t : w * 2 * T + T + t + 1],
                            op0=mybir.AluOpType.is_equal,
                            op1=mybir.AluOpType.mult,
                        ).then_inc(SEM_S, 1)
                    # copy window aggregate out of PSUM
                    ab = (W % 2) * f
                    vector.wait_ge(SEM_MM, (W + 1) * T)
                    vector.tensor_copy(
                        agg_sb[:, ab : ab + f], agg_ps[W % 2][:, :]
                    ).then_inc(SEM_CP, 1)
                    # copy transposes out of PSUM
                    for k in range(kf):
                        vector.wait_ge(SEM_TP, kf * W + k + 1)
                        vector.tensor_copy(
                            tp_sb[:, k * P : (k + 1) * P], tp_ps[k][:, :]
                        ).then_inc(SEM_TPC, 1)
                    # copy final output out of PSUM
                    ob = (W % 2) * f
                    if W >= 2:
                        vector.wait_ge(SEM_OD, (W - 1) * 16)
                    vector.wait_ge(SEM_FIN, W + 1)
                    vector.tensor_copy(
                        out_sb[:, ob : ob + f], out_ps[W % 2][:, :]
                    ).then_inc(SEM_OUT, 1)

            @block.tensor
            def _(tensor):
                tensor.wait_ge(SEM_META, 80)
                tensor.wait_ge(SEM_CONST, 1)
                for W in range(repeat * wpc):
                    w = W % wpc
                    ab = (W % 2) * f
                    gb = (W % NG) * T * f
                    if W >= 2:
                        tensor.wait_ge(SEM_CP, W - 1)  # agg bank free
                    tensor.wait_ge(SEM_S, (W + 1) * T)  # all S of window ready
                    for t in range(T):
                        i = W * T + t
                        sb = (i % NS) * P
                        tensor.wait_ge(SEM_G, 16 * (i + 1))  # tile t gathered
                        tensor.matmul(
                            agg_ps[W % 2][:, :],
                            s_buf[:, sb : sb + P],
                            g_buf[:, gb + t * f : gb + (t + 1) * f],
                            start=(t == 0),
                            stop=(t == T - 1),
                        ).then_inc(SEM_MM, 1)
                    tensor.wait_ge(SEM_CP, W + 1)  # agg_sb ready
                    for k in range(kf):
                        if W >= 1:
                            tensor.wait_ge(SEM_TPC, kf * (W - 1) + k + 1)  # tp bank free
                        tensor.transpose(
                            tp_ps[k][:, :],
                            agg_sb[:, ab + k * P : ab + (k + 1) * P],
                            ident[:, :],
                        ).then_inc(SEM_TP, 1)
                    ob = (W % 2) * f
                    if W >= 2:
                        tensor.wait_ge(SEM_OUT, W - 1)  # out_ps bank free
                    for k in range(kf):
                        tensor.wait_ge(SEM_TPC, kf * W + k + 1)  # tp_sb ready
                        tensor.matmul(
                            out_ps[W % 2][:, :],
                            tp_sb[:, k * P : (k + 1) * P],
                            w_sb[k][:, :],
                            start=(k == 0),
                            stop=False,
                        )
                    tensor.wait_ge(SEM_MSK, 16 * (W + 1))
                    tensor.matmul(
                        out_ps[W % 2][:, :],
                        msk_t[:1, (W % 2) * P : (W % 2) * P + P],
                        b_sb[:1, :],
                        start=False,
                        stop=True,
                    ).then_inc(SEM_FIN, 1)

    return nc


def prepare_inputs(x, edge_index, W, b, n_cores=N_CORES):
    """Host-side: sort/bucket edges by destination into per-core padded windows."""
    n = x.shape[0]
    npc = n // n_cores
    wpc = (npc + P - 1) // P

    row = np.asarray(edge_index[0], dtype=np.int64)  # dest
    col = np.asarray(edge_index[1], dtype=np.int64)  # src

    deg = np.bincount(row, minlength=n).astype(np.float32)
    invdeg = np.zeros(n, dtype=np.float32)
    nz = deg > 0
    invdeg[nz] = 1.0 / deg[nz]

    order = np.argsort(row, kind="stable")
    row_s = row[order]
    col_s = col[order]

    core_of = row_s // npc
    local = row_s - core_of * npc
    win = local // P
    dstl = local % P
    gwin = core_of * wpc + win
    n_gw = n_cores * wpc

    counts = np.bincount(gwin, minlength=n_gw)
    n_tiles = max(1, int(np.ceil(counts.max() / P)))
    T = n_tiles

    first = np.searchsorted(gwin, np.arange(n_gw))
    pos = np.arange(len(gwin)) - first[gwin]
    t_of = pos // P
    p_of = pos % P

    srcidx = np.zeros((n_cores, wpc, P, T), dtype=np.int32)
    dstloc = np.full((n_cores, wpc, P, 2 * T), -1.0, dtype=np.float32)

    srcidx[core_of, win, p_of, t_of] = col_s.astype(np.int32)
    dstloc[core_of, win, p_of, t_of] = dstl.astype(np.float32)
    dstloc[core_of, win, p_of, T + t_of] = invdeg[row_s]

    maskw = np.zeros((n_cores, wpc * P), dtype=np.float32)
    maskw[:, :npc] = nz.astype(np.float32).reshape(n_cores, npc)
    maskw = maskw.reshape(n_cores, wpc, P)

    x_c = np.ascontiguousarray(x, dtype=mybir.dt.np(mybir.dt.float32))
    per_core = []
    for c in range(n_cores):
        per_core.append(
            {
                "x": x_c,
                "srcidx": np.ascontiguousarray(
                    srcidx[c].transpose(1, 0, 2).reshape(P, wpc * T)
                ),
                "dw": np.ascontiguousarray(
                    dstloc[c].transpose(1, 0, 2).reshape(P, wpc * 2 * T)
                ),
                "maskw": maskw[c],
                "Wm": np.ascontiguousarray(W, dtype=np.float32),
                "bv": np.ascontiguousarray(b, dtype=np.float32).reshape(1, -1),
            }
        )
    return per_core, n_tiles


def run(x, edge_index, W, b, n_cores=N_CORES, trace=False):
    n, f = x.shape
    npc = n // n_cores
    in_maps, n_tiles = prepare_inputs(x, edge_index, W, b, n_cores)
    nc = build_nc(n, npc, n_tiles)
    res = run_bass_kernel_spmd(nc, in_maps, list(range(n_cores)), trace=trace)
    out = np.concatenate([res.results[c]["out"] for c in range(n_cores)], axis=0)
    return out, res


def kernel(x, edge_index, W, b):
    out, _ = run(np.asarray(x), np.asarray(edge_index), np.asarray(W), np.asarray(b))
    return out.astype(np.float32)



# revision 7
# speedup vs baseline: 1.5216x; 1.5216x over previous
"""GraphSAGE mean-aggregation layer on 8 Trainium2 NeuronCores (Bass/Bacc).

Math: out = D^{-1} A (x @ W + b)  ==  (D^{-1} A x) @ W + mask (outer) b
where A is the (row=dest, col=src) adjacency from edge_index, D = row degrees,
mask[d] = 1 if deg[d] > 0 else 0 (zero-degree rows are exactly 0 in the ref).

Strategy (one SPMD program on 8 cores, dest nodes sharded, bf16 data path):
  - Host: sort edges by dest, bucket into 128-dest windows (wpc per core).
    Within a window, edges are grouped by source QUADRANT (4 tables of
    25000 rows each so indices fit int16 for dma_gather), each group padded
    to the global per-quadrant tile count T_q with (idx=0, weight=0) slots.
    Per-edge weight 1/deg[dest] is folded into the one-hot S matrix.
  - Device, per window: 8 dma_gather calls (per quadrant: one 8-tile call +
    one (T_q-8)-tile call, capped at 1024 rows each by the SWDGE ring),
    round-robined over 4 SWDGE queues whose descriptor generation runs in
    parallel. Then per 128-edge tile a DVE-built weighted one-hot S (bf16)
    and a PE matmul S^T @ G accumulating into PSUM [128 dests, 256] fp32;
    transpose (bf16) + W matmul (bf16) + masked bias, DMA 128 rows out.
  - Raw bass engine programs with explicit semaphores; one sync wait per
    instruction (standalone wait_ge).
"""

import numpy as np
import ml_dtypes

import concourse.bass as bass
import concourse.bacc as bacc
import concourse.mybir as mybir
from concourse.bass_utils import run_bass_kernel_spmd
from concourse.library_config import mlp

P = 128
F = 256

N_NODES = 100000
N_CORES = 8
NPC = N_NODES // N_CORES  # dest rows per core
NQ = 4                    # source quadrant tables
QROWS = N_NODES // NQ     # rows per quadrant table (int16-addressable)
CAP_TILES = 8             # SWDGE ring: <=1024 descriptors per dma_gather call

BF16 = mybir.dt.bfloat16


def build_nc(npc, t_q):
    """One SPMD Bass program; t_q = tiles per (window, quadrant)."""
    wpc = (npc + P - 1) // P
    T = NQ * t_q  # edge tiles per 128-dest window
    f = F
    kf = f // P
    dt_f32 = mybir.dt.float32

    # per-quadrant gather calls: chunks of <= CAP_TILES tiles
    chunks = []
    t0 = 0
    while t0 < t_q:
        chunks.append((t0, min(CAP_TILES, t_q - t0)))
        t0 += CAP_TILES
    cols_per_q = t_q * P // 16     # idx table columns per (window, quadrant)
    cols_per_w = NQ * cols_per_q

    nc = bacc.Bacc("TRN2", num_swdge_queues=4)

    xq_h = [
        nc.declare_dram_parameter(f"x{q}", [QROWS, f], BF16, isOutput=False)
        for q in range(NQ)
    ]
    idx_h = nc.declare_dram_parameter(
        "srcidx", [P, wpc * cols_per_w], mybir.dt.int16, isOutput=False
    )
    dw_h = nc.declare_dram_parameter("dw", [P, wpc * 2 * T], dt_f32, isOutput=False)
    msk_h = nc.declare_dram_parameter("maskw", [wpc, P], BF16, isOutput=False)
    w_h = nc.declare_dram_parameter("Wm", [f, f], BF16, isOutput=False)
    b_h = nc.declare_dram_parameter("bv", [1, f], BF16, isOutput=False)
    out_h = nc.declare_dram_parameter("out", [npc, f], dt_f32, isOutput=True)

    NS = T + 12  # S-tile ring: one window + pipeline margin

    from contextlib import ExitStack

    ctx = ExitStack()
    with ctx:
        sb = lambda name, shape, dt: ctx.enter_context(nc.sbuf_tensor(name, shape, dt))
        ps = lambda name, shape, dt: ctx.enter_context(nc.psum_tensor(name, shape, dt))
        sem = lambda name: ctx.enter_context(nc.semaphore(name))

        iota_f = sb("iota_f", [P, P], BF16)
        ident = sb("ident", [P, P], BF16)
        w0 = sb("w0", [P, f], BF16)
        w1 = sb("w1", [P, f], BF16)
        b_sb = sb("b_sb", [1, f], BF16)
        idx_all = sb("idx_all", [P, wpc * cols_per_w], mybir.dt.int16)
        dw_all = sb("dw_all", [P, wpc * 2 * T], dt_f32)
        msk_t = sb("msk_t", [1, 2 * P], BF16)
        g_buf = sb("g_buf", [P, 2, T, f], BF16)
        s_buf = sb("s_buf", [P, NS, P], BF16)
        agg_sb = sb("agg_sb", [P, 2 * f], BF16)
        tp_sb = sb("tp_sb", [P, kf * P], BF16)
        out_sb = sb("out_sb", [P, 2 * f], dt_f32)
        agg_ps = [ps("agg_ps0", [P, f], dt_f32), ps("agg_ps1", [P, f], dt_f32)]
        tp_ps = [ps("tp_ps0", [P, P], BF16), ps("tp_ps1", [P, P], BF16)]
        out_ps = [ps("out_ps0", [P, f], dt_f32), ps("out_ps1", [P, f], dt_f32)]
        SEM_META = sem("sem_meta")
        SEM_CONST = sem("sem_const")
        SEM_GW = [[sem(f"sem_gw{par}q{q}") for q in range(NQ)] for par in range(2)]
        SEM_S = sem("sem_s")
        SEM_MM = sem("sem_mm")
        SEM_CP = sem("sem_cp")
        SEM_TP = sem("sem_tp")
        SEM_TPC = sem("sem_tpc")
        SEM_FIN = sem("sem_fin")
        SEM_OUT = sem("sem_out")
        SEM_OD = sem("sem_od")
        SEM_MSK = sem("sem_msk")

        w_sb = [w0, w1]
        calls_per_w = NQ * len(chunks)

        with nc.Block() as block:

            @block.sync
            def _(sync):
                # startup loads (HWDGE)
                sync.dma_start(w0[:, :], w_h[0:P, :]).then_inc(SEM_META, 16)
                sync.dma_start(w1[:, :], w_h[P : 2 * P, :]).then_inc(SEM_META, 16)
                sync.dma_start(b_sb[:, :], b_h[:, :]).then_inc(SEM_META, 16)
                sync.dma_start(idx_all[:, :], idx_h[:, :]).then_inc(SEM_META, 16)
                sync.dma_start(dw_all[:, :], dw_h[:, :]).then_inc(SEM_META, 16)
                # per-window mask loads + output stores
                for W in range(wpc):
                    rows = min(P, npc - W * P)
                    ob = (W % 2) * f
                    mb = (W % 2) * P
                    if W >= 2:
                        sync.wait_ge(SEM_FIN, W - 1)  # msk_t slot free
                    sync.dma_start(
                        msk_t[:1, mb : mb + P], msk_h[W : W + 1, :]
                    ).then_inc(SEM_MSK, 16)
                    sync.wait_ge(SEM_OUT, W + 1)
                    sync.dma_start(
                        out_h[W * P : W * P + rows, :], out_sb[:rows, ob : ob + f]
                    ).then_inc(SEM_OD, 16)

            @block.gpsimd
            def _(gpsimd):
                gpsimd.load_library(mlp)
                # constants (gpsimd ops are unordered across DSP cores: sync each)
                gpsimd.iota(
                    iota_f[:, :],
                    pattern=[[1, P]],
                    base=0,
                    channel_multiplier=0,
                    allow_small_or_imprecise_dtypes=True,
                ).then_inc(SEM_CONST, 1)
                gpsimd.memset(ident[:, :], 0.0).then_inc(SEM_CONST, 1)
                gpsimd.wait_ge(SEM_CONST, 2)
                gpsimd.affine_select(
                    out=ident[:, :],
                    in_=ident[:, :],
                    compare_op=mybir.AluOpType.not_equal,
                    fill=1.0,
                    base=0,
                    pattern=[[-1, P]],
                    channel_multiplier=1,
                ).then_inc(SEM_CONST, 1)
                nregs = {nt: gpsimd.to_reg(nt * P) for _, nt in set(chunks)}
                # gathers
                gpsimd.wait_ge(SEM_META, 80)
                for W in range(wpc):
                    par = W % 2
                    if W >= 2:
                        # g_buf parity slot free once window W-2 is consumed
                        gpsimd.wait_ge(SEM_MM, (W - 1) * T)
                    for q in range(NQ):
                        cbase = W * cols_per_w + q * cols_per_q
                        for c0, nt in chunks:
                            gpsimd.dma_gather(
                                g_buf[:, par, q * t_q + c0 : q * t_q + c0 + nt, :],
                                xq_h[q][:, :],
                                idx_all[
                                    :,
                                    cbase + c0 * 8 : cbase + (c0 + nt) * 8,
                                ],
                                nt * P,
                                nregs[nt],
                                f,
                                queue_num=q,
                            ).then_inc(SEM_GW[par][q], 16)

            @block.vector
            def _(vector):
                vector.wait_ge(SEM_CONST, 3)
                vector.wait_ge(SEM_META, 80)
                for W in range(wpc):
                    # build S tiles for window W
                    for t in range(T):
                        i = W * T + t
                        sslot = (i % NS) * P
                        if i >= NS:
                            vector.wait_ge(SEM_MM, i - NS + 1)
                        vector.tensor_scalar(
                            out=s_buf[:, i % NS, :],
                            in0=iota_f[:, :],
                            scalar1=dw_all[:, W * 2 * T + t : W * 2 * T + t + 1],
                            scalar2=dw_all[:, W * 2 * T + T + t : W * 2 * T + T + t + 1],
                            op0=mybir.AluOpType.is_equal,
                            op1=mybir.AluOpType.mult,
                        ).then_inc(SEM_S, 1)
                    # copy window aggregate out of PSUM (f32 -> bf16)
                    ab = (W % 2) * f
                    vector.wait_ge(SEM_MM, (W + 1) * T)
                    vector.tensor_copy(
                        agg_sb[:, ab : ab + f], agg_ps[W % 2][:, :]
                    ).then_inc(SEM_CP, 1)
                    # copy transposes out of PSUM
                    for k in range(kf):
                        vector.wait_ge(SEM_TP, kf * W + k + 1)
                        vector.tensor_copy(
                            tp_sb[:, k * P : (k + 1) * P], tp_ps[k][:, :]
                        ).then_inc(SEM_TPC, 1)
                    # copy final output out of PSUM
                    ob = (W % 2) * f
                    if W >= 2:
                        vector.wait_ge(SEM_OD, (W - 1) * 16)
                    vector.wait_ge(SEM_FIN, W + 1)
                    vector.tensor_copy(
                        out_sb[:, ob : ob + f], out_ps[W % 2][:, :]
                    ).then_inc(SEM_OUT, 1)

            @block.tensor
            def _(tensor):
                tensor.wait_ge(SEM_META, 80)
                tensor.wait_ge(SEM_CONST, 3)
                for W in range(wpc):
                    par = W % 2
                    ab = par * f
                    if W >= 2:
                        tensor.wait_ge(SEM_CP, W - 1)  # agg bank free
                    tensor.wait_ge(SEM_S, (W + 1) * T)  # all S of window ready
                    for q in range(NQ):  # window gathered (per queue)
                        tensor.wait_ge(
                            SEM_GW[par][q], 16 * len(chunks) * (W // 2 + 1)
                        )
                    for t in range(T):
                        i = W * T + t
                        tensor.matmul(
                            agg_ps[par][:, :],
                            s_buf[:, i % NS, :],
                            g_buf[:, par, t, :],
                            start=(t == 0),
                            stop=(t == T - 1),
                        ).then_inc(SEM_MM, 1)
                    tensor.wait_ge(SEM_CP, W + 1)  # agg_sb ready
                    for k in range(kf):
                        if W >= 1:
                            tensor.wait_ge(SEM_TPC, kf * (W - 1) + k + 1)  # tp bank free
                        tensor.transpose(
                            tp_ps[k][:, :],
                            agg_sb[:, ab + k * P : ab + (k + 1) * P],
                            ident[:, :],
                        ).then_inc(SEM_TP, 1)
                    ob = par * f
                    if W >= 2:
                        tensor.wait_ge(SEM_OUT, W - 1)  # out_ps bank free
                    for k in range(kf):
                        tensor.wait_ge(SEM_TPC, kf * W + k + 1)  # tp_sb ready
                        tensor.matmul(
                            out_ps[par][:, :],
                            tp_sb[:, k * P : (k + 1) * P],
                            w_sb[k][:, :],
                            start=(k == 0),
                            stop=False,
                        )
                    tensor.wait_ge(SEM_MSK, 16 * (W + 1))
                    tensor.matmul(
                        out_ps[par][:, :],
                        msk_t[:1, par * P : par * P + P],
                        b_sb[:1, :],
                        start=False,
                        stop=True,
                    ).then_inc(SEM_FIN, 1)

    nc.compile()
    return nc


def prepare_inputs(x, edge_index, W, b, n_cores=N_CORES):
    """Host-side: sort/bucket edges by destination into per-core padded windows,
    grouped by source quadrant."""
    n = x.shape[0]
    npc = n // n_cores
    wpc = (npc + P - 1) // P

    row = np.asarray(edge_index[0], dtype=np.int64)  # dest
    col = np.asarray(edge_index[1], dtype=np.int64)  # src

    deg = np.bincount(row, minlength=n).astype(np.float64)
    invdeg = np.zeros(n, dtype=np.float64)
    nz = deg > 0
    invdeg[nz] = 1.0 / deg[nz]

    # group edges by (core, window, src-quadrant); order within a run is free
    # (each edge carries its own dest label into the one-hot S matrix)
    quad = col // QROWS
    core0 = row // npc
    win0 = (row - core0 * npc) // P
    order = np.lexsort((quad, core0 * wpc + win0))
    row_s = row[order]
    col_s = col[order]
    quad_s = quad[order]

    core_of = row_s // npc
    local = row_s - core_of * npc
    win = local // P
    gslot = ((core_of * wpc) + win) * NQ + quad_s  # global (window, quadrant) run

    n_runs = n_cores * wpc * NQ
    counts = np.bincount(gslot, minlength=n_runs)
    t_q = max(1, int(np.ceil(counts.max() / P)))

    first = np.searchsorted(gslot, np.arange(n_runs))
    pos = np.arange(len(gslot)) - first[gslot]  # position within the run

    T = NQ * t_q
    rows_q = t_q * P

    # per (core, window, quadrant): padded run of rows_q edge slots
    # srcidx layout: [core][window][quadrant][wrapped16: (col, p16)]
    srcidx = np.zeros((n_cores, wpc, NQ, rows_q), dtype=np.int16)
    dst_t = np.full((n_cores, wpc, T, P), -1.0, dtype=np.float32)
    wgt_t = np.zeros((n_cores, wpc, T, P), dtype=np.float32)

    src_local = (col_s - quad_s * QROWS).astype(np.int16)
    srcidx[core_of, win, quad_s, pos] = src_local
    tile = quad_s * t_q + pos // P
    dst_t[core_of, win, tile, pos % P] = (local % P).astype(np.float32)
    wgt_t[core_of, win, tile, pos % P] = invdeg[row_s]

    # wrap-16: in-run row j -> (partition j%16, column j//16), tiled to 128
    srcidx = srcidx.reshape(n_cores, wpc, NQ, rows_q // 16, 16)
    srcidx = np.transpose(srcidx, (0, 4, 1, 2, 3))  # [cores, 16, wpc, NQ, cols]
    srcidx = np.tile(srcidx, (1, 8, 1, 1, 1)).reshape(n_cores, P, -1)

    # dw layout: [P, wpc * 2T]: per window w: cols [w*2T, w*2T+T) = dst,
    # [w*2T+T, (w+1)*2T) = weight; per tile, edges on partitions.
    dw = np.concatenate([dst_t, wgt_t], axis=2)  # [cores, wpc, 2T, P]
    dw = np.transpose(dw, (0, 3, 1, 2)).reshape(n_cores, P, wpc * 2 * T)

    maskw = np.zeros((n_cores, wpc * P), dtype=np.float32)
    maskw[:, :npc] = nz.astype(np.float32).reshape(n_cores, npc)
    maskw = maskw.reshape(n_cores, wpc, P).astype(ml_dtypes.bfloat16)

    x_bf = np.ascontiguousarray(np.asarray(x, dtype=np.float32)).astype(
        ml_dtypes.bfloat16
    )
    xq = [np.ascontiguousarray(x_bf[q * QROWS : (q + 1) * QROWS]) for q in range(NQ)]
    W_bf = np.ascontiguousarray(np.asarray(W, dtype=np.float32)).astype(
        ml_dtypes.bfloat16
    )
    b_bf = (
        np.ascontiguousarray(np.asarray(b, dtype=np.float32))
        .astype(ml_dtypes.bfloat16)
        .reshape(1, -1)
    )

    per_core = []
    for c in range(n_cores):
        m = {f"x{q}": xq[q] for q in range(NQ)}
        m["srcidx"] = np.ascontiguousarray(srcidx[c])
        m["dw"] = np.ascontiguousarray(dw[c])
        m["maskw"] = maskw[c]
        m["Wm"] = W_bf
        m["bv"] = b_bf
        per_core.append(m)
    return per_core, t_q


def run(x, edge_index, W, b, n_cores=N_CORES, trace=False):
    n, f = x.shape
    npc = n // n_cores
    in_maps, t_q = prepare_inputs(x, edge_index, W, b, n_cores)
    nc = build_nc(npc, t_q)
    res = run_bass_kernel_spmd(nc, in_maps, list(range(n_cores)), trace=trace)
    out = np.concatenate([res.results[c]["out"] for c in range(n_cores)], axis=0)
    return out, res


def kernel(x, edge_index, W, b):
    out, _ = run(np.asarray(x), np.asarray(edge_index), np.asarray(W), np.asarray(b))
    return out.astype(np.float32)


# revision 8
# speedup vs baseline: 1.5975x; 1.0499x over previous
"""GraphSAGE mean-aggregation layer on 8 Trainium2 NeuronCores (Bass/Bacc).

Math: out = D^{-1} A (x @ W + b)  ==  (D^{-1} A x) @ W + mask (outer) b
where A is the (row=dest, col=src) adjacency from edge_index, D = row degrees,
mask[d] = 1 if deg[d] > 0 else 0 (zero-degree rows are exactly 0 in the ref).

Strategy (one SPMD program on 8 cores, dest nodes sharded, bf16 data path):
  - Host: sort edges by dest, bucket into 128-dest windows (wpc per core).
    Within a window, edges are grouped by source QUADRANT (4 tables of
    25000 rows each so indices fit int16 for dma_gather), each group padded
    to the global per-quadrant tile count T_q with (idx=0, weight=0) slots.
    Per-edge weight 1/deg[dest] is folded into the one-hot S matrix.
  - Device, per window: 8 dma_gather calls (per quadrant: one 8-tile call +
    one (T_q-8)-tile call, capped at 1024 rows each by the SWDGE ring),
    round-robined over 4 SWDGE queues whose descriptor generation runs in
    parallel. Then per 128-edge tile a DVE-built weighted one-hot S (bf16)
    and a PE matmul S^T @ G accumulating into PSUM [128 dests, 256] fp32;
    transpose (bf16) + W matmul (bf16) + masked bias, DMA 128 rows out.
  - Raw bass engine programs with explicit semaphores; one sync wait per
    instruction (standalone wait_ge).
"""

import numpy as np
import ml_dtypes

import concourse.bass as bass
import concourse.bacc as bacc
import concourse.mybir as mybir
from concourse.bass_utils import run_bass_kernel_spmd
from concourse.library_config import mlp

P = 128
F = 256

N_NODES = 100000
N_CORES = 8
NPC = N_NODES // N_CORES  # dest rows per core
NQ = 4                    # source quadrant tables
QROWS = N_NODES // NQ     # rows per quadrant table (int16-addressable)
CAP_TILES = 8             # SWDGE ring: <=1024 descriptors per dma_gather call

BF16 = mybir.dt.bfloat16


def build_nc(npc, t_q):
    """One SPMD Bass program; t_q = tiles per (window, quadrant)."""
    wpc = (npc + P - 1) // P
    T = NQ * t_q  # edge tiles per 128-dest window
    f = F
    kf = f // P
    dt_f32 = mybir.dt.float32

    # per-quadrant gather calls: chunks of <= CAP_TILES tiles
    chunks = []
    t0 = 0
    while t0 < t_q:
        chunks.append((t0, min(CAP_TILES, t_q - t0)))
        t0 += CAP_TILES
    cols_per_q = t_q * P // 16     # idx table columns per (window, quadrant)
    cols_per_w = NQ * cols_per_q

    nc = bacc.Bacc("TRN2", num_swdge_queues=4)

    xq_h = [
        nc.declare_dram_parameter(f"x{q}", [QROWS, f], BF16, isOutput=False)
        for q in range(NQ)
    ]
    idx_h = nc.declare_dram_parameter(
        "srcidx", [P, wpc * cols_per_w], mybir.dt.int16, isOutput=False
    )
    dw_h = nc.declare_dram_parameter("dw", [P, wpc * 2 * T], dt_f32, isOutput=False)
    msk_h = nc.declare_dram_parameter("maskw", [wpc, P], BF16, isOutput=False)
    w_h = nc.declare_dram_parameter("Wm", [f, f], BF16, isOutput=False)
    b_h = nc.declare_dram_parameter("bv", [1, f], BF16, isOutput=False)
    out_h = nc.declare_dram_parameter("out", [npc, f], dt_f32, isOutput=True)

    NS = T + 12  # S-tile ring: one window + pipeline margin

    from contextlib import ExitStack

    ctx = ExitStack()
    with ctx:
        sb = lambda name, shape, dt: ctx.enter_context(nc.sbuf_tensor(name, shape, dt))
        ps = lambda name, shape, dt: ctx.enter_context(nc.psum_tensor(name, shape, dt))
        sem = lambda name: ctx.enter_context(nc.semaphore(name))

        iota_f = sb("iota_f", [P, P], BF16)
        ident = sb("ident", [P, P], BF16)
        w0 = sb("w0", [P, f], BF16)
        w1 = sb("w1", [P, f], BF16)
        b_sb = sb("b_sb", [1, f], BF16)
        idx_all = sb("idx_all", [P, wpc * cols_per_w], mybir.dt.int16)
        dw_all = sb("dw_all", [P, wpc * 2 * T], dt_f32)
        msk_t = sb("msk_t", [1, 2 * P], BF16)
        g_buf = sb("g_buf", [P, 2, T, f], BF16)
        s_buf = sb("s_buf", [P, NS, P], BF16)
        agg_sb = sb("agg_sb", [P, 2 * f], BF16)
        tp_sb = sb("tp_sb", [P, kf * P], BF16)
        out_sb = sb("out_sb", [P, 2 * f], dt_f32)
        agg_ps = [ps("agg_ps0", [P, f], dt_f32), ps("agg_ps1", [P, f], dt_f32)]
        tp_ps = [ps("tp_ps0", [P, P], BF16), ps("tp_ps1", [P, P], BF16)]
        out_ps = [ps("out_ps0", [P, f], dt_f32), ps("out_ps1", [P, f], dt_f32)]
        SEM_META = sem("sem_meta")
        SEM_CONST = sem("sem_const")
        SEM_GW = [[sem(f"sem_gw{par}q{q}") for q in range(NQ)] for par in range(2)]
        SEM_S = sem("sem_s")
        SEM_MM = sem("sem_mm")
        SEM_CP = sem("sem_cp")
        SEM_TP = sem("sem_tp")
        SEM_TPC = sem("sem_tpc")
        SEM_FIN = sem("sem_fin")
        SEM_OUT = sem("sem_out")
        SEM_OD = sem("sem_od")
        SEM_MSK = sem("sem_msk")

        w_sb = [w0, w1]
        calls_per_w = NQ * len(chunks)

        with nc.Block() as block:

            @block.sync
            def _(sync):
                # startup loads (HWDGE)
                sync.dma_start(w0[:, :], w_h[0:P, :]).then_inc(SEM_META, 16)
                sync.dma_start(w1[:, :], w_h[P : 2 * P, :]).then_inc(SEM_META, 16)
                sync.dma_start(b_sb[:, :], b_h[:, :]).then_inc(SEM_META, 16)
                sync.dma_start(idx_all[:, :], idx_h[:, :]).then_inc(SEM_META, 16)
                sync.dma_start(dw_all[:, :], dw_h[:, :]).then_inc(SEM_META, 16)
                # per-window mask loads + output stores
                for W in range(wpc):
                    rows = min(P, npc - W * P)
                    ob = (W % 2) * f
                    mb = (W % 2) * P
                    if W >= 2:
                        sync.wait_ge(SEM_FIN, W - 1)  # msk_t slot free
                    sync.dma_start(
                        msk_t[:1, mb : mb + P], msk_h[W : W + 1, :]
                    ).then_inc(SEM_MSK, 16)
                    sync.wait_ge(SEM_OUT, W + 1)
                    sync.dma_start(
                        out_h[W * P : W * P + rows, :], out_sb[:rows, ob : ob + f]
                    ).then_inc(SEM_OD, 16)

            @block.gpsimd
            def _(gpsimd):
                gpsimd.load_library(mlp)
                # constants (gpsimd ops are unordered across DSP cores: sync each)
                gpsimd.iota(
                    iota_f[:, :],
                    pattern=[[1, P]],
                    base=0,
                    channel_multiplier=0,
                    allow_small_or_imprecise_dtypes=True,
                ).then_inc(SEM_CONST, 1)
                gpsimd.memset(ident[:, :], 0.0).then_inc(SEM_CONST, 1)
                gpsimd.wait_ge(SEM_CONST, 2)
                gpsimd.affine_select(
                    out=ident[:, :],
                    in_=ident[:, :],
                    compare_op=mybir.AluOpType.not_equal,
                    fill=1.0,
                    base=0,
                    pattern=[[-1, P]],
                    channel_multiplier=1,
                ).then_inc(SEM_CONST, 1)
                nregs = {nt: gpsimd.to_reg(nt * P) for _, nt in set(chunks)}
                # gathers
                gpsimd.wait_ge(SEM_META, 80)
                for W in range(wpc):
                    par = W % 2
                    if W >= 2:
                        # g_buf parity slot free once window W-2 is consumed
                        gpsimd.wait_ge(SEM_MM, (W - 1) * T)
                    for q in range(NQ):
                        cbase = W * cols_per_w + q * cols_per_q
                        for c0, nt in chunks:
                            gpsimd.dma_gather(
                                g_buf[:, par, q * t_q + c0 : q * t_q + c0 + nt, :],
                                xq_h[q][:, :],
                                idx_all[
                                    :,
                                    cbase + c0 * 8 : cbase + (c0 + nt) * 8,
                                ],
                                nt * P,
                                nregs[nt],
                                f,
                                queue_num=q,
                            ).then_inc(SEM_GW[par][q], 16)

            @block.vector
            def _(vector):
                vector.wait_ge(SEM_CONST, 3)
                vector.wait_ge(SEM_META, 80)
                for W in range(wpc):
                    # build S tiles for window W
                    for t in range(T):
                        i = W * T + t
                        sslot = (i % NS) * P
                        if i >= NS:
                            vector.wait_ge(SEM_MM, i - NS + 1)
                        vector.tensor_scalar(
                            out=s_buf[:, i % NS, :],
                            in0=iota_f[:, :],
                            scalar1=dw_all[:, W * 2 * T + t : W * 2 * T + t + 1],
                            scalar2=dw_all[:, W * 2 * T + T + t : W * 2 * T + T + t + 1],
                            op0=mybir.AluOpType.is_equal,
                            op1=mybir.AluOpType.mult,
                        ).then_inc(SEM_S, 1)
                    # copy window aggregate out of PSUM (f32 -> bf16)
                    ab = (W % 2) * f
                    vector.wait_ge(SEM_MM, (W + 1) * T)
                    vector.tensor_copy(
                        agg_sb[:, ab : ab + f], agg_ps[W % 2][:, :]
                    ).then_inc(SEM_CP, 1)
                    # copy transposes out of PSUM
                    for k in range(kf):
                        vector.wait_ge(SEM_TP, kf * W + k + 1)
                        vector.tensor_copy(
                            tp_sb[:, k * P : (k + 1) * P], tp_ps[k][:, :]
                        ).then_inc(SEM_TPC, 1)
                    # copy final output out of PSUM
                    ob = (W % 2) * f
                    if W >= 2:
                        vector.wait_ge(SEM_OD, (W - 1) * 16)
                    vector.wait_ge(SEM_FIN, W + 1)
                    vector.tensor_copy(
                        out_sb[:, ob : ob + f], out_ps[W % 2][:, :]
                    ).then_inc(SEM_OUT, 1)

            @block.tensor
            def _(tensor):
                tensor.wait_ge(SEM_META, 80)
                tensor.wait_ge(SEM_CONST, 3)
                for W in range(wpc):
                    par = W % 2
                    ab = par * f
                    if W >= 2:
                        tensor.wait_ge(SEM_CP, W - 1)  # agg bank free
                    tensor.wait_ge(SEM_S, (W + 1) * T)  # all S of window ready
                    for q in range(NQ):  # window gathered (per queue)
                        tensor.wait_ge(
                            SEM_GW[par][q], 16 * len(chunks) * (W // 2 + 1)
                        )
                    for t in range(T):
                        i = W * T + t
                        tensor.matmul(
                            agg_ps[par][:, :],
                            s_buf[:, i % NS, :],
                            g_buf[:, par, t, :],
                            start=(t == 0),
                            stop=(t == T - 1),
                        ).then_inc(SEM_MM, 1)
                    tensor.wait_ge(SEM_CP, W + 1)  # agg_sb ready
                    for k in range(kf):
                        if W >= 1:
                            tensor.wait_ge(SEM_TPC, kf * (W - 1) + k + 1)  # tp bank free
                        tensor.transpose(
                            tp_ps[k][:, :],
                            agg_sb[:, ab + k * P : ab + (k + 1) * P],
                            ident[:, :],
                        ).then_inc(SEM_TP, 1)
                    ob = par * f
                    if W >= 2:
                        tensor.wait_ge(SEM_OUT, W - 1)  # out_ps bank free
                    for k in range(kf):
                        tensor.wait_ge(SEM_TPC, kf * W + k + 1)  # tp_sb ready
                        tensor.matmul(
                            out_ps[par][:, :],
                            tp_sb[:, k * P : (k + 1) * P],
                            w_sb[k][:, :],
                            start=(k == 0),
                            stop=False,
                        )
                    tensor.wait_ge(SEM_MSK, 16 * (W + 1))
                    tensor.matmul(
                        out_ps[par][:, :],
                        msk_t[:1, par * P : par * P + P],
                        b_sb[:1, :],
                        start=False,
                        stop=True,
                    ).then_inc(SEM_FIN, 1)

    nc.compile()
    return nc


def prepare_inputs(x, edge_index, W, b, n_cores=N_CORES):
    """Host-side: sort/bucket edges by destination into per-core padded windows,
    grouped by source quadrant."""
    n = x.shape[0]
    npc = n // n_cores
    wpc = (npc + P - 1) // P

    row = np.asarray(edge_index[0], dtype=np.int64)  # dest
    col = np.asarray(edge_index[1], dtype=np.int64)  # src

    deg = np.bincount(row, minlength=n).astype(np.float64)
    invdeg = np.zeros(n, dtype=np.float64)
    nz = deg > 0
    invdeg[nz] = 1.0 / deg[nz]

    # group edges by (core, window, src-quadrant); order within a run is free
    # (each edge carries its own dest label into the one-hot S matrix)
    quad = col // QROWS
    core0 = row // npc
    win0 = (row - core0 * npc) // P
    order = np.lexsort((quad, core0 * wpc + win0))
    row_s = row[order]
    col_s = col[order]
    quad_s = quad[order]

    core_of = row_s // npc
    local = row_s - core_of * npc
    win = local // P
    gslot = ((core_of * wpc) + win) * NQ + quad_s  # global (window, quadrant) run

    n_runs = n_cores * wpc * NQ
    counts = np.bincount(gslot, minlength=n_runs)
    t_q = max(1, int(np.ceil(counts.max() / P)))

    first = np.searchsorted(gslot, np.arange(n_runs))
    pos = np.arange(len(gslot)) - first[gslot]  # position within the run

    T = NQ * t_q
    rows_q = t_q * P

    # per (core, window, quadrant): padded run of rows_q edge slots
    # srcidx layout: [core][window][quadrant][wrapped16: (col, p16)]
    # pad slots carry weight 0 but still gather a row: spread them across the
    # table so duplicate-address reads don't serialize on one HBM line
    pad_rows = ((np.arange(rows_q, dtype=np.int64) * 97) % QROWS).astype(np.int16)
    srcidx = np.broadcast_to(
        pad_rows, (n_cores, wpc, NQ, rows_q)
    ).copy()
    dst_t = np.full((n_cores, wpc, T, P), -1.0, dtype=np.float32)
    wgt_t = np.zeros((n_cores, wpc, T, P), dtype=np.float32)

    src_local = (col_s - quad_s * QROWS).astype(np.int16)
    srcidx[core_of, win, quad_s, pos] = src_local
    tile = quad_s * t_q + pos // P
    dst_t[core_of, win, tile, pos % P] = (local % P).astype(np.float32)
    wgt_t[core_of, win, tile, pos % P] = invdeg[row_s]

    # wrap-16: in-run row j -> (partition j%16, column j//16), tiled to 128
    srcidx = srcidx.reshape(n_cores, wpc, NQ, rows_q // 16, 16)
    srcidx = np.transpose(srcidx, (0, 4, 1, 2, 3))  # [cores, 16, wpc, NQ, cols]
    srcidx = np.tile(srcidx, (1, 8, 1, 1, 1)).reshape(n_cores, P, -1)

    # dw layout: [P, wpc * 2T]: per window w: cols [w*2T, w*2T+T) = dst,
    # [w*2T+T, (w+1)*2T) = weight; per tile, edges on partitions.
    dw = np.concatenate([dst_t, wgt_t], axis=2)  # [cores, wpc, 2T, P]
    dw = np.transpose(dw, (0, 3, 1, 2)).reshape(n_cores, P, wpc * 2 * T)

    maskw = np.zeros((n_cores, wpc * P), dtype=np.float32)
    maskw[:, :npc] = nz.astype(np.float32).reshape(n_cores, npc)
    maskw = maskw.reshape(n_cores, wpc, P).astype(ml_dtypes.bfloat16)

    x_bf = np.ascontiguousarray(np.asarray(x, dtype=np.float32)).astype(
        ml_dtypes.bfloat16
    )
    xq = [np.ascontiguousarray(x_bf[q * QROWS : (q + 1) * QROWS]) for q in range(NQ)]
    W_bf = np.ascontiguousarray(np.asarray(W, dtype=np.float32)).astype(
        ml_dtypes.bfloat16
    )
    b_bf = (
        np.ascontiguousarray(np.asarray(b, dtype=np.float32))
        .astype(ml_dtypes.bfloat16)
        .reshape(1, -1)
    )

    per_core = []
    for c in range(n_cores):
        m = {f"x{q}": xq[q] for q in range(NQ)}
        m["srcidx"] = np.ascontiguousarray(srcidx[c])
        m["dw"] = np.ascontiguousarray(dw[c])
        m["maskw"] = maskw[c]
        m["Wm"] = W_bf
        m["bv"] = b_bf
        per_core.append(m)
    return per_core, t_q


def run(x, edge_index, W, b, n_cores=N_CORES, trace=False):
    n, f = x.shape
    npc = n // n_cores
    in_maps, t_q = prepare_inputs(x, edge_index, W, b, n_cores)
    nc = build_nc(npc, t_q)
    res = run_bass_kernel_spmd(nc, in_maps, list(range(n_cores)), trace=trace)
    out = np.concatenate([res.results[c]["out"] for c in range(n_cores)], axis=0)
    return out, res


def kernel(x, edge_index, W, b):
    out, _ = run(np.asarray(x), np.asarray(edge_index), np.asarray(W), np.asarray(b))
    return out.astype(np.float32)


# revision 9
# speedup vs baseline: 1.8518x; 1.1592x over previous
"""GraphSAGE mean-aggregation layer on 8 Trainium2 NeuronCores (Bass/Bacc).

Math: out = D^{-1} A (x @ W + b)  ==  (D^{-1} A x) @ W + mask (outer) b
where A is the (row=dest, col=src) adjacency from edge_index, D = row degrees,
mask[d] = 1 if deg[d] > 0 else 0 (zero-degree rows are exactly 0 in the ref).

Strategy (one SPMD program on 8 cores, dest nodes sharded, bf16 data path):
  - Host: sort edges by dest, bucket into 128-dest windows (wpc per core).
    Within a window, edges are grouped by source QUADRANT (4 tables of
    25000 rows each so indices fit int16 for dma_gather), each group padded
    to the global per-quadrant tile count T_q with (idx=0, weight=0) slots.
    Per-edge weight 1/deg[dest] is folded into the one-hot S matrix.
  - Device, per window: 8 dma_gather calls (per quadrant: one 8-tile call +
    one (T_q-8)-tile call, capped at 1024 rows each by the SWDGE ring),
    round-robined over 4 SWDGE queues whose descriptor generation runs in
    parallel. Then per 128-edge tile a DVE-built weighted one-hot S (bf16)
    and a PE matmul S^T @ G accumulating into PSUM [128 dests, 256] fp32;
    transpose (bf16) + W matmul (bf16) + masked bias, DMA 128 rows out.
  - Raw bass engine programs with explicit semaphores; one sync wait per
    instruction (standalone wait_ge).
"""

import numpy as np
import ml_dtypes

import concourse.bass as bass
import concourse.bacc as bacc
import concourse.mybir as mybir
from concourse.bass_utils import run_bass_kernel_spmd
from concourse.library_config import mlp

P = 128
F = 256

N_NODES = 100000
N_CORES = 8
NPC = N_NODES // N_CORES  # dest rows per core
NQ = 4                    # source quadrant tables
QROWS = N_NODES // NQ     # rows per quadrant table (int16-addressable)
CAP_TILES = 8             # SWDGE ring: <=1024 descriptors per dma_gather call

BF16 = mybir.dt.bfloat16


def build_nc(npc, t_q):
    """One SPMD Bass program; t_q = tiles per (window, quadrant)."""
    wpc = (npc + P - 1) // P
    T = NQ * t_q  # edge tiles per 128-dest window
    f = F
    kf = f // P
    dt_f32 = mybir.dt.float32

    # per-quadrant gather calls: chunks of <= CAP_TILES tiles
    chunks = []
    t0 = 0
    while t0 < t_q:
        chunks.append((t0, min(CAP_TILES, t_q - t0)))
        t0 += CAP_TILES
    cols_per_q = t_q * P // 16     # idx table columns per (window, quadrant)
    cols_per_w = NQ * cols_per_q

    nc = bacc.Bacc("TRN2", num_swdge_queues=4)

    xq_h = [
        nc.declare_dram_parameter(f"x{q}", [QROWS, f], BF16, isOutput=False)
        for q in range(NQ)
    ]
    idx_h = nc.declare_dram_parameter(
        "srcidx", [P, wpc * cols_per_w], mybir.dt.int16, isOutput=False
    )
    dw_h = nc.declare_dram_parameter("dw", [P, wpc * 2 * T], dt_f32, isOutput=False)
    msk_h = nc.declare_dram_parameter("maskw", [wpc, P], BF16, isOutput=False)
    w_h = nc.declare_dram_parameter("Wm", [f, f], BF16, isOutput=False)
    b_h = nc.declare_dram_parameter("bv", [1, f], BF16, isOutput=False)
    out_h = nc.declare_dram_parameter("out", [npc, f], dt_f32, isOutput=True)

    NS = 2 * T + 16  # S-tile ring: ~two windows + pipeline margin

    from contextlib import ExitStack

    ctx = ExitStack()
    with ctx:
        sb = lambda name, shape, dt: ctx.enter_context(nc.sbuf_tensor(name, shape, dt))
        ps = lambda name, shape, dt: ctx.enter_context(nc.psum_tensor(name, shape, dt))
        sem = lambda name: ctx.enter_context(nc.semaphore(name))

        iota_f = sb("iota_f", [P, P], BF16)
        ident = sb("ident", [P, P], BF16)
        w0 = sb("w0", [P, f], BF16)
        w1 = sb("w1", [P, f], BF16)
        b_sb = sb("b_sb", [1, f], BF16)
        idx_all = sb("idx_all", [P, wpc * cols_per_w], mybir.dt.int16)
        dw_all = sb("dw_all", [P, wpc * 2 * T], dt_f32)
        msk_t = sb("msk_t", [1, 2 * P], BF16)
        g_buf = sb("g_buf", [P, 2, T, f], BF16)
        s_buf = sb("s_buf", [P, NS, P], BF16)
        agg_sb = sb("agg_sb", [P, 2 * f], BF16)
        tp_sb = sb("tp_sb", [P, kf * P], BF16)
        out_sb = sb("out_sb", [P, 2 * f], dt_f32)
        agg_ps = [ps("agg_ps0", [P, f], dt_f32), ps("agg_ps1", [P, f], dt_f32)]
        tp_ps = [ps("tp_ps0", [P, P], BF16), ps("tp_ps1", [P, P], BF16)]
        out_ps = [ps("out_ps0", [P, f], dt_f32), ps("out_ps1", [P, f], dt_f32)]
        SEM_META = sem("sem_meta")
        SEM_CONST = sem("sem_const")
        SEM_GW = [[sem(f"sem_gw{par}q{q}") for q in range(NQ)] for par in range(2)]
        SEM_S = sem("sem_s")
        SEM_MM = sem("sem_mm")
        SEM_CP = sem("sem_cp")
        SEM_TP = sem("sem_tp")
        SEM_TPC = sem("sem_tpc")
        SEM_FIN = sem("sem_fin")
        SEM_OUT = sem("sem_out")
        SEM_OD = sem("sem_od")
        SEM_MSK = sem("sem_msk")

        w_sb = [w0, w1]
        calls_per_w = NQ * len(chunks)

        with nc.Block() as block:

            @block.sync
            def _(sync):
                # startup loads (HWDGE)
                sync.dma_start(w0[:, :], w_h[0:P, :]).then_inc(SEM_META, 16)
                sync.dma_start(w1[:, :], w_h[P : 2 * P, :]).then_inc(SEM_META, 16)
                sync.dma_start(b_sb[:, :], b_h[:, :]).then_inc(SEM_META, 16)
                sync.dma_start(idx_all[:, :], idx_h[:, :]).then_inc(SEM_META, 16)
                sync.dma_start(dw_all[:, :], dw_h[:, :]).then_inc(SEM_META, 16)
                # per-window mask loads + output stores
                for W in range(wpc):
                    rows = min(P, npc - W * P)
                    ob = (W % 2) * f
                    mb = (W % 2) * P
                    if W >= 2:
                        sync.wait_ge(SEM_FIN, W - 1)  # msk_t slot free
                    sync.dma_start(
                        msk_t[:1, mb : mb + P], msk_h[W : W + 1, :]
                    ).then_inc(SEM_MSK, 16)
                    sync.wait_ge(SEM_OUT, W + 1)
                    sync.dma_start(
                        out_h[W * P : W * P + rows, :], out_sb[:rows, ob : ob + f]
                    ).then_inc(SEM_OD, 16)

            @block.gpsimd
            def _(gpsimd):
                gpsimd.load_library(mlp)
                # constants (gpsimd ops are unordered across DSP cores: sync each)
                gpsimd.iota(
                    iota_f[:, :],
                    pattern=[[1, P]],
                    base=0,
                    channel_multiplier=0,
                    allow_small_or_imprecise_dtypes=True,
                ).then_inc(SEM_CONST, 1)
                gpsimd.memset(ident[:, :], 0.0).then_inc(SEM_CONST, 1)
                gpsimd.wait_ge(SEM_CONST, 2)
                gpsimd.affine_select(
                    out=ident[:, :],
                    in_=ident[:, :],
                    compare_op=mybir.AluOpType.not_equal,
                    fill=1.0,
                    base=0,
                    pattern=[[-1, P]],
                    channel_multiplier=1,
                ).then_inc(SEM_CONST, 1)
                nregs = {nt: gpsimd.to_reg(nt * P) for _, nt in set(chunks)}
                # gathers
                gpsimd.wait_ge(SEM_META, 80)
                for W in range(wpc):
                    par = W % 2
                    if W >= 2:
                        # g_buf parity slot free once window W-2 is consumed
                        gpsimd.wait_ge(SEM_MM, (W - 1) * T)
                    for q in range(NQ):
                        cbase = W * cols_per_w + q * cols_per_q
                        for c0, nt in chunks:
                            gpsimd.dma_gather(
                                g_buf[:, par, q * t_q + c0 : q * t_q + c0 + nt, :],
                                xq_h[q][:, :],
                                idx_all[
                                    :,
                                    cbase + c0 * 8 : cbase + (c0 + nt) * 8,
                                ],
                                nt * P,
                                nregs[nt],
                                f,
                                queue_num=q,
                            ).then_inc(SEM_GW[par][q], 16)

            @block.vector
            def _(vector):
                vector.wait_ge(SEM_CONST, 3)
                vector.wait_ge(SEM_META, 80)
                for W in range(wpc):
                    # build S tiles for window W
                    for t in range(T):
                        i = W * T + t
                        sslot = (i % NS) * P
                        if i >= NS:
                            vector.wait_ge(SEM_MM, i - NS + 1)
                        vector.tensor_scalar(
                            out=s_buf[:, i % NS, :],
                            in0=iota_f[:, :],
                            scalar1=dw_all[:, W * 2 * T + t : W * 2 * T + t + 1],
                            scalar2=dw_all[:, W * 2 * T + T + t : W * 2 * T + T + t + 1],
                            op0=mybir.AluOpType.is_equal,
                            op1=mybir.AluOpType.mult,
                        ).then_inc(SEM_S, 1)
                    # copy window aggregate out of PSUM (f32 -> bf16)
                    ab = (W % 2) * f
                    vector.wait_ge(SEM_MM, (W + 1) * T)
                    vector.tensor_copy(
                        agg_sb[:, ab : ab + f], agg_ps[W % 2][:, :]
                    ).then_inc(SEM_CP, 1)
                    # copy transposes out of PSUM
                    for k in range(kf):
                        vector.wait_ge(SEM_TP, kf * W + k + 1)
                        vector.tensor_copy(
                            tp_sb[:, k * P : (k + 1) * P], tp_ps[k][:, :]
                        ).then_inc(SEM_TPC, 1)
                    # copy final output out of PSUM
                    ob = (W % 2) * f
                    if W >= 2:
                        vector.wait_ge(SEM_OD, (W - 1) * 16)
                    vector.wait_ge(SEM_FIN, W + 1)
                    vector.tensor_copy(
                        out_sb[:, ob : ob + f], out_ps[W % 2][:, :]
                    ).then_inc(SEM_OUT, 1)

            @block.tensor
            def _(tensor):
                tensor.wait_ge(SEM_META, 80)
                tensor.wait_ge(SEM_CONST, 3)
                for W in range(wpc):
                    par = W % 2
                    ab = par * f
                    if W >= 2:
                        tensor.wait_ge(SEM_CP, W - 1)  # agg bank free
                    for t in range(T):
                        i = W * T + t
                        if t % t_q == 0:  # quadrant gathered (per queue)
                            tensor.wait_ge(
                                SEM_GW[par][t // t_q],
                                16 * len(chunks) * (W // 2 + 1),
                            )
                        tensor.wait_ge(SEM_S, i + 1)  # this S tile built
                        tensor.matmul(
                            agg_ps[par][:, :],
                            s_buf[:, i % NS, :],
                            g_buf[:, par, t, :],
                            start=(t == 0),
                            stop=(t == T - 1),
                        ).then_inc(SEM_MM, 1)
                    tensor.wait_ge(SEM_CP, W + 1)  # agg_sb ready
                    for k in range(kf):
                        if W >= 1:
                            tensor.wait_ge(SEM_TPC, kf * (W - 1) + k + 1)  # tp bank free
                        tensor.transpose(
                            tp_ps[k][:, :],
                            agg_sb[:, ab + k * P : ab + (k + 1) * P],
                            ident[:, :],
                        ).then_inc(SEM_TP, 1)
                    ob = par * f
                    if W >= 2:
                        tensor.wait_ge(SEM_OUT, W - 1)  # out_ps bank free
                    for k in range(kf):
                        tensor.wait_ge(SEM_TPC, kf * W + k + 1)  # tp_sb ready
                        tensor.matmul(
                            out_ps[par][:, :],
                            tp_sb[:, k * P : (k + 1) * P],
                            w_sb[k][:, :],
                            start=(k == 0),
                            stop=False,
                        )
                    tensor.wait_ge(SEM_MSK, 16 * (W + 1))
                    tensor.matmul(
                        out_ps[par][:, :],
                        msk_t[:1, par * P : par * P + P],
                        b_sb[:1, :],
                        start=False,
                        stop=True,
                    ).then_inc(SEM_FIN, 1)

    nc.compile()
    return nc


def prepare_inputs(x, edge_index, W, b, n_cores=N_CORES):
    """Host-side: sort/bucket edges by destination into per-core padded windows,
    grouped by source quadrant."""
    n = x.shape[0]
    npc = n // n_cores
    wpc = (npc + P - 1) // P

    row = np.asarray(edge_index[0], dtype=np.int64)  # dest
    col = np.asarray(edge_index[1], dtype=np.int64)  # src

    deg = np.bincount(row, minlength=n).astype(np.float64)
    invdeg = np.zeros(n, dtype=np.float64)
    nz = deg > 0
    invdeg[nz] = 1.0 / deg[nz]

    # group edges by (core, window, src-quadrant); order within a run is free
    # (each edge carries its own dest label into the one-hot S matrix)
    quad = col // QROWS
    core0 = row // npc
    win0 = (row - core0 * npc) // P
    order = np.lexsort((quad, core0 * wpc + win0))
    row_s = row[order]
    col_s = col[order]
    quad_s = quad[order]

    core_of = row_s // npc
    local = row_s - core_of * npc
    win = local // P
    gslot = ((core_of * wpc) + win) * NQ + quad_s  # global (window, quadrant) run

    n_runs = n_cores * wpc * NQ
    counts = np.bincount(gslot, minlength=n_runs)
    t_q = max(1, int(np.ceil(counts.max() / P)))

    first = np.searchsorted(gslot, np.arange(n_runs))
    pos = np.arange(len(gslot)) - first[gslot]  # position within the run

    T = NQ * t_q
    rows_q = t_q * P

    # per (core, window, quadrant): padded run of rows_q edge slots
    # srcidx layout: [core][window][quadrant][wrapped16: (col, p16)]
    # pad slots carry weight 0 but still gather a row: spread them across the
    # table so duplicate-address reads don't serialize on one HBM line
    pad_rows = ((np.arange(rows_q, dtype=np.int64) * 97) % QROWS).astype(np.int16)
    srcidx = np.broadcast_to(
        pad_rows, (n_cores, wpc, NQ, rows_q)
    ).copy()
    dst_t = np.full((n_cores, wpc, T, P), -1.0, dtype=np.float32)
    wgt_t = np.zeros((n_cores, wpc, T, P), dtype=np.float32)

    src_local = (col_s - quad_s * QROWS).astype(np.int16)
    srcidx[core_of, win, quad_s, pos] = src_local
    tile = quad_s * t_q + pos // P
    dst_t[core_of, win, tile, pos % P] = (local % P).astype(np.float32)
    wgt_t[core_of, win, tile, pos % P] = invdeg[row_s]

    # wrap-16: in-run row j -> (partition j%16, column j//16), tiled to 128
    srcidx = srcidx.reshape(n_cores, wpc, NQ, rows_q // 16, 16)
    srcidx = np.transpose(srcidx, (0, 4, 1, 2, 3))  # [cores, 16, wpc, NQ, cols]
    srcidx = np.tile(srcidx, (1, 8, 1, 1, 1)).reshape(n_cores, P, -1)

    # dw layout: [P, wpc * 2T]: per window w: cols [w*2T, w*2T+T) = dst,
    # [w*2T+T, (w+1)*2T) = weight; per tile, edges on partitions.
    dw = np.concatenate([dst_t, wgt_t], axis=2)  # [cores, wpc, 2T, P]
    dw = np.transpose(dw, (0, 3, 1, 2)).reshape(n_cores, P, wpc * 2 * T)

    maskw = np.zeros((n_cores, wpc * P), dtype=np.float32)
    maskw[:, :npc] = nz.astype(np.float32).reshape(n_cores, npc)
    maskw = maskw.reshape(n_cores, wpc, P).astype(ml_dtypes.bfloat16)

    x_bf = np.ascontiguousarray(np.asarray(x, dtype=np.float32)).astype(
        ml_dtypes.bfloat16
    )
    xq = [np.ascontiguousarray(x_bf[q * QROWS : (q + 1) * QROWS]) for q in range(NQ)]
    W_bf = np.ascontiguousarray(np.asarray(W, dtype=np.float32)).astype(
        ml_dtypes.bfloat16
    )
    b_bf = (
        np.ascontiguousarray(np.asarray(b, dtype=np.float32))
        .astype(ml_dtypes.bfloat16)
        .reshape(1, -1)
    )

    per_core = []
    for c in range(n_cores):
        m = {f"x{q}": xq[q] for q in range(NQ)}
        m["srcidx"] = np.ascontiguousarray(srcidx[c])
        m["dw"] = np.ascontiguousarray(dw[c])
        m["maskw"] = maskw[c]
        m["Wm"] = W_bf
        m["bv"] = b_bf
        per_core.append(m)
    return per_core, t_q


def run(x, edge_index, W, b, n_cores=N_CORES, trace=False):
    n, f = x.shape
    npc = n // n_cores
    in_maps, t_q = prepare_inputs(x, edge_index, W, b, n_cores)
    nc = build_nc(npc, t_q)
    res = run_bass_kernel_spmd(nc, in_maps, list(range(n_cores)), trace=trace)
    out = np.concatenate([res.results[c]["out"] for c in range(n_cores)], axis=0)
    return out, res


def kernel(x, edge_index, W, b):
    out, _ = run(np.asarray(x), np.asarray(edge_index), np.asarray(W), np.asarray(b))
    return out.astype(np.float32)


# revision 10
# speedup vs baseline: 1.8523x; 1.0003x over previous
"""GraphSAGE mean-aggregation layer on 8 Trainium2 NeuronCores (Bass/Bacc).

Math: out = D^{-1} A (x @ W + b)  ==  (D^{-1} A x) @ W + mask (outer) b
where A is the (row=dest, col=src) adjacency from edge_index, D = row degrees,
mask[d] = 1 if deg[d] > 0 else 0 (zero-degree rows are exactly 0 in the ref).

Strategy (one SPMD program on 8 cores, dest nodes sharded, bf16 data path):
  - Host: sort edges by dest, bucket into 128-dest windows (wpc per core).
    Within a window, edges are grouped by source QUADRANT (4 tables of
    25000 rows each so indices fit int16 for dma_gather), each group padded
    to the global per-quadrant tile count T_q with (idx=0, weight=0) slots.
    Per-edge weight 1/deg[dest] is folded into the one-hot S matrix.
  - Device, per window: 8 dma_gather calls (per quadrant: one 8-tile call +
    one (T_q-8)-tile call, capped at 1024 rows each by the SWDGE ring),
    round-robined over 4 SWDGE queues whose descriptor generation runs in
    parallel. Then per 128-edge tile a DVE-built weighted one-hot S (bf16)
    and a PE matmul S^T @ G accumulating into PSUM [128 dests, 256] fp32;
    transpose (bf16) + W matmul (bf16) + masked bias, DMA 128 rows out.
  - Raw bass engine programs with explicit semaphores; one sync wait per
    instruction (standalone wait_ge).
"""

import numpy as np
import ml_dtypes

import concourse.bass as bass
import concourse.bacc as bacc
import concourse.mybir as mybir
from concourse.bass_utils import run_bass_kernel_spmd
from concourse.library_config import mlp

P = 128
F = 256

N_NODES = 100000
N_CORES = 8
NPC = N_NODES // N_CORES  # dest rows per core
NQ = 4                    # source quadrant tables
QROWS = N_NODES // NQ     # rows per quadrant table (int16-addressable)
CAP_TILES = 8             # SWDGE ring: <=1024 descriptors per dma_gather call

BF16 = mybir.dt.bfloat16


def build_nc(npc, t_q):
    """One SPMD Bass program; t_q = tiles per (window, quadrant)."""
    wpc = (npc + P - 1) // P
    T = NQ * t_q  # edge tiles per 128-dest window
    f = F
    kf = f // P
    dt_f32 = mybir.dt.float32

    # per-quadrant gather calls: chunks of <= CAP_TILES tiles
    chunks = []
    t0 = 0
    while t0 < t_q:
        chunks.append((t0, min(CAP_TILES, t_q - t0)))
        t0 += CAP_TILES
    cols_per_q = t_q * P // 16     # idx table columns per (window, quadrant)
    cols_per_w = NQ * cols_per_q

    nc = bacc.Bacc("TRN2", num_swdge_queues=4)

    xq_h = [
        nc.declare_dram_parameter(f"x{q}", [QROWS, f], BF16, isOutput=False)
        for q in range(NQ)
    ]
    idx_h = nc.declare_dram_parameter(
        "srcidx", [P, wpc * cols_per_w], mybir.dt.int16, isOutput=False
    )
    dw_h = nc.declare_dram_parameter("dw", [P, wpc * 2 * T], dt_f32, isOutput=False)
    msk_h = nc.declare_dram_parameter("maskw", [wpc, P], BF16, isOutput=False)
    w_h = nc.declare_dram_parameter("Wm", [f, f], BF16, isOutput=False)
    b_h = nc.declare_dram_parameter("bv", [1, f], BF16, isOutput=False)
    out_h = nc.declare_dram_parameter("out", [npc, f], dt_f32, isOutput=True)


    from contextlib import ExitStack

    ctx = ExitStack()
    with ctx:
        sb = lambda name, shape, dt: ctx.enter_context(nc.sbuf_tensor(name, shape, dt))
        ps = lambda name, shape, dt: ctx.enter_context(nc.psum_tensor(name, shape, dt))
        sem = lambda name: ctx.enter_context(nc.semaphore(name))

        iota_f = sb("iota_f", [P, P], BF16)
        ident = sb("ident", [P, P], BF16)
        w0 = sb("w0", [P, f], BF16)
        w1 = sb("w1", [P, f], BF16)
        b_sb = sb("b_sb", [1, f], BF16)
        idx_all = sb("idx_all", [P, wpc * cols_per_w], mybir.dt.int16)
        dw_all = sb("dw_all", [P, wpc * 2 * T], dt_f32)
        msk_t = sb("msk_t", [1, 2 * P], BF16)
        g_buf = sb("g_buf", [P, 2, T, f], BF16)
        s_buf = sb("s_buf", [P, 2, T, P], BF16)
        agg_sb = sb("agg_sb", [P, 2 * f], BF16)
        tp_sb = sb("tp_sb", [P, kf * P], BF16)
        out_sb = sb("out_sb", [P, 2 * f], dt_f32)
        agg_ps = [ps("agg_ps0", [P, f], dt_f32), ps("agg_ps1", [P, f], dt_f32)]
        tp_ps = [ps("tp_ps0", [P, P], BF16), ps("tp_ps1", [P, P], BF16)]
        out_ps = [ps("out_ps0", [P, f], dt_f32), ps("out_ps1", [P, f], dt_f32)]
        SEM_META = sem("sem_meta")
        SEM_CONST = sem("sem_const")
        SEM_GW = [[sem(f"sem_gw{par}q{q}") for q in range(NQ)] for par in range(2)]
        SEM_S = sem("sem_s")
        SEM_MM = sem("sem_mm")
        SEM_CP = sem("sem_cp")
        SEM_TP = sem("sem_tp")
        SEM_TPC = sem("sem_tpc")
        SEM_FIN = sem("sem_fin")
        SEM_OUT = sem("sem_out")
        SEM_OD = sem("sem_od")
        SEM_MSK = sem("sem_msk")

        w_sb = [w0, w1]
        calls_per_w = NQ * len(chunks)

        with nc.Block() as block:

            @block.sync
            def _(sync):
                # startup loads (HWDGE)
                sync.dma_start(w0[:, :], w_h[0:P, :]).then_inc(SEM_META, 16)
                sync.dma_start(w1[:, :], w_h[P : 2 * P, :]).then_inc(SEM_META, 16)
                sync.dma_start(b_sb[:, :], b_h[:, :]).then_inc(SEM_META, 16)
                sync.dma_start(idx_all[:, :], idx_h[:, :]).then_inc(SEM_META, 16)
                sync.dma_start(dw_all[:, :], dw_h[:, :]).then_inc(SEM_META, 16)
                # per-window mask loads + output stores
                for W in range(wpc):
                    rows = min(P, npc - W * P)
                    ob = (W % 2) * f
                    mb = (W % 2) * P
                    if W >= 2:
                        sync.wait_ge(SEM_FIN, W - 1)  # msk_t slot free
                    sync.dma_start(
                        msk_t[:1, mb : mb + P], msk_h[W : W + 1, :]
                    ).then_inc(SEM_MSK, 16)
                    sync.wait_ge(SEM_OUT, W + 1)
                    sync.dma_start(
                        out_h[W * P : W * P + rows, :], out_sb[:rows, ob : ob + f]
                    ).then_inc(SEM_OD, 16)

            @block.gpsimd
            def _(gpsimd):
                gpsimd.load_library(mlp)
                # constants (gpsimd ops are unordered across DSP cores: sync each)
                gpsimd.iota(
                    iota_f[:, :],
                    pattern=[[1, P]],
                    base=0,
                    channel_multiplier=0,
                    allow_small_or_imprecise_dtypes=True,
                ).then_inc(SEM_CONST, 1)
                gpsimd.memset(ident[:, :], 0.0).then_inc(SEM_CONST, 1)
                gpsimd.wait_ge(SEM_CONST, 2)
                gpsimd.affine_select(
                    out=ident[:, :],
                    in_=ident[:, :],
                    compare_op=mybir.AluOpType.not_equal,
                    fill=1.0,
                    base=0,
                    pattern=[[-1, P]],
                    channel_multiplier=1,
                ).then_inc(SEM_CONST, 1)
                nregs = {nt: gpsimd.to_reg(nt * P) for _, nt in set(chunks)}
                # gathers
                gpsimd.wait_ge(SEM_META, 80)
                for W in range(wpc):
                    par = W % 2
                    if W >= 2:
                        # g_buf parity slot free once window W-2 is consumed
                        gpsimd.wait_ge(SEM_MM, (W - 1) * T)
                    for q in range(NQ):
                        cbase = W * cols_per_w + q * cols_per_q
                        for c0, nt in chunks:
                            gpsimd.dma_gather(
                                g_buf[:, par, q * t_q + c0 : q * t_q + c0 + nt, :],
                                xq_h[q][:, :],
                                idx_all[
                                    :,
                                    cbase + c0 * 8 : cbase + (c0 + nt) * 8,
                                ],
                                nt * P,
                                nregs[nt],
                                f,
                                queue_num=q,
                            ).then_inc(SEM_GW[par][q], 16)

            @block.vector
            def _(vector):
                vector.wait_ge(SEM_CONST, 3)
                vector.wait_ge(SEM_META, 80)
                for W in range(wpc):
                    # build S tiles for window W (parity half W%2; free once
                    # window W-2's matmuls are done)
                    if W >= 2:
                        vector.wait_ge(SEM_MM, (W - 1) * T)
                    for t in range(T):
                        i = W * T + t
                        vector.tensor_scalar(
                            out=s_buf[:, W % 2, t, :],
                            in0=iota_f[:, :],
                            scalar1=dw_all[:, W * 2 * T + t : W * 2 * T + t + 1],
                            scalar2=dw_all[:, W * 2 * T + T + t : W * 2 * T + T + t + 1],
                            op0=mybir.AluOpType.is_equal,
                            op1=mybir.AluOpType.mult,
                        ).then_inc(SEM_S, 1)
                    # copy window aggregate out of PSUM (f32 -> bf16)
                    ab = (W % 2) * f
                    vector.wait_ge(SEM_MM, (W + 1) * T)
                    vector.tensor_copy(
                        agg_sb[:, ab : ab + f], agg_ps[W % 2][:, :]
                    ).then_inc(SEM_CP, 1)
                    # copy transposes out of PSUM
                    for k in range(kf):
                        vector.wait_ge(SEM_TP, kf * W + k + 1)
                        vector.tensor_copy(
                            tp_sb[:, k * P : (k + 1) * P], tp_ps[k][:, :]
                        ).then_inc(SEM_TPC, 1)
                    # copy final output out of PSUM
                    ob = (W % 2) * f
                    if W >= 2:
                        vector.wait_ge(SEM_OD, (W - 1) * 16)
                    vector.wait_ge(SEM_FIN, W + 1)
                    vector.tensor_copy(
                        out_sb[:, ob : ob + f], out_ps[W % 2][:, :]
                    ).then_inc(SEM_OUT, 1)

            @block.tensor
            def _(tensor):
                tensor.wait_ge(SEM_META, 80)
                tensor.wait_ge(SEM_CONST, 3)
                for W in range(wpc):
                    par = W % 2
                    ab = par * f
                    if W >= 2:
                        tensor.wait_ge(SEM_CP, W - 1)  # agg bank free
                    for t in range(T):
                        i = W * T + t
                        if t % t_q == 0:  # quadrant gathered (per queue)
                            tensor.wait_ge(
                                SEM_GW[par][t // t_q],
                                16 * len(chunks) * (W // 2 + 1),
                            )
                        tensor.wait_ge(SEM_S, i + 1)  # this S tile built
                        tensor.matmul(
                            agg_ps[par][:, :],
                            s_buf[:, par, t, :],
                            g_buf[:, par, t, :],
                            start=(t == 0),
                            stop=(t == T - 1),
                        ).then_inc(SEM_MM, 1)
                    tensor.wait_ge(SEM_CP, W + 1)  # agg_sb ready
                    for k in range(kf):
                        if W >= 1:
                            tensor.wait_ge(SEM_TPC, kf * (W - 1) + k + 1)  # tp bank free
                        tensor.transpose(
                            tp_ps[k][:, :],
                            agg_sb[:, ab + k * P : ab + (k + 1) * P],
                            ident[:, :],
                        ).then_inc(SEM_TP, 1)
                    ob = par * f
                    if W >= 2:
                        tensor.wait_ge(SEM_OUT, W - 1)  # out_ps bank free
                    for k in range(kf):
                        tensor.wait_ge(SEM_TPC, kf * W + k + 1)  # tp_sb ready
                        tensor.matmul(
                            out_ps[par][:, :],
                            tp_sb[:, k * P : (k + 1) * P],
                            w_sb[k][:, :],
                            start=(k == 0),
                            stop=False,
                        )
                    tensor.wait_ge(SEM_MSK, 16 * (W + 1))
                    tensor.matmul(
                        out_ps[par][:, :],
                        msk_t[:1, par * P : par * P + P],
                        b_sb[:1, :],
                        start=False,
                        stop=True,
                    ).then_inc(SEM_FIN, 1)

    nc.compile()
    return nc


def prepare_inputs(x, edge_index, W, b, n_cores=N_CORES):
    """Host-side: sort/bucket edges by destination into per-core padded windows,
    grouped by source quadrant."""
    n = x.shape[0]
    npc = n // n_cores
    wpc = (npc + P - 1) // P

    row = np.asarray(edge_index[0], dtype=np.int64)  # dest
    col = np.asarray(edge_index[1], dtype=np.int64)  # src

    deg = np.bincount(row, minlength=n).astype(np.float64)
    invdeg = np.zeros(n, dtype=np.float64)
    nz = deg > 0
    invdeg[nz] = 1.0 / deg[nz]

    # group edges by (core, window, src-quadrant); order within a run is free
    # (each edge carries its own dest label into the one-hot S matrix)
    quad = col // QROWS
    core0 = row // npc
    win0 = (row - core0 * npc) // P
    order = np.lexsort((quad, core0 * wpc + win0))
    row_s = row[order]
    col_s = col[order]
    quad_s = quad[order]

    core_of = row_s // npc
    local = row_s - core_of * npc
    win = local // P
    gslot = ((core_of * wpc) + win) * NQ + quad_s  # global (window, quadrant) run

    n_runs = n_cores * wpc * NQ
    counts = np.bincount(gslot, minlength=n_runs)
    t_q = max(1, int(np.ceil(counts.max() / P)))

    first = np.searchsorted(gslot, np.arange(n_runs))
    pos = np.arange(len(gslot)) - first[gslot]  # position within the run

    T = NQ * t_q
    rows_q = t_q * P

    # per (core, window, quadrant): padded run of rows_q edge slots
    # srcidx layout: [core][window][quadrant][wrapped16: (col, p16)]
    # pad slots carry weight 0 but still gather a row: spread them across the
    # table so duplicate-address reads don't serialize on one HBM line
    pad_rows = ((np.arange(rows_q, dtype=np.int64) * 97) % QROWS).astype(np.int16)
    srcidx = np.broadcast_to(
        pad_rows, (n_cores, wpc, NQ, rows_q)
    ).copy()
    dst_t = np.full((n_cores, wpc, T, P), -1.0, dtype=np.float32)
    wgt_t = np.zeros((n_cores, wpc, T, P), dtype=np.float32)

    src_local = (col_s - quad_s * QROWS).astype(np.int16)
    srcidx[core_of, win, quad_s, pos] = src_local
    tile = quad_s * t_q + pos // P
    dst_t[core_of, win, tile, pos % P] = (local % P).astype(np.float32)
    wgt_t[core_of, win, tile, pos % P] = invdeg[row_s]

    # wrap-16: in-run row j -> (partition j%16, column j//16), tiled to 128
    srcidx = srcidx.reshape(n_cores, wpc, NQ, rows_q // 16, 16)
    srcidx = np.transpose(srcidx, (0, 4, 1, 2, 3))  # [cores, 16, wpc, NQ, cols]
    srcidx = np.tile(srcidx, (1, 8, 1, 1, 1)).reshape(n_cores, P, -1)

    # dw layout: [P, wpc * 2T]: per window w: cols [w*2T, w*2T+T) = dst,
    # [w*2T+T, (w+1)*2T) = weight; per tile, edges on partitions.
    dw = np.concatenate([dst_t, wgt_t], axis=2)  # [cores, wpc, 2T, P]
    dw = np.transpose(dw, (0, 3, 1, 2)).reshape(n_cores, P, wpc * 2 * T)

    maskw = np.zeros((n_cores, wpc * P), dtype=np.float32)
    maskw[:, :npc] = nz.astype(np.float32).reshape(n_cores, npc)
    maskw = maskw.reshape(n_cores, wpc, P).astype(ml_dtypes.bfloat16)

    x_bf = np.ascontiguousarray(np.asarray(x, dtype=np.float32)).astype(
        ml_dtypes.bfloat16
    )
    xq = [np.ascontiguousarray(x_bf[q * QROWS : (q + 1) * QROWS]) for q in range(NQ)]
    W_bf = np.ascontiguousarray(np.asarray(W, dtype=np.float32)).astype(
        ml_dtypes.bfloat16
    )
    b_bf = (
        np.ascontiguousarray(np.asarray(b, dtype=np.float32))
        .astype(ml_dtypes.bfloat16)
        .reshape(1, -1)
    )

    per_core = []
    for c in range(n_cores):
        m = {f"x{q}": xq[q] for q in range(NQ)}
        m["srcidx"] = np.ascontiguousarray(srcidx[c])
        m["dw"] = np.ascontiguousarray(dw[c])
        m["maskw"] = maskw[c]
        m["Wm"] = W_bf
        m["bv"] = b_bf
        per_core.append(m)
    return per_core, t_q


def run(x, edge_index, W, b, n_cores=N_CORES, trace=False):
    n, f = x.shape
    npc = n // n_cores
    in_maps, t_q = prepare_inputs(x, edge_index, W, b, n_cores)
    nc = build_nc(npc, t_q)
    res = run_bass_kernel_spmd(nc, in_maps, list(range(n_cores)), trace=trace)
    out = np.concatenate([res.results[c]["out"] for c in range(n_cores)], axis=0)
    return out, res


def kernel(x, edge_index, W, b):
    out, _ = run(np.asarray(x), np.asarray(edge_index), np.asarray(W), np.asarray(b))
    return out.astype(np.float32)


# revision 13
# speedup vs baseline: 3.0938x; 1.6702x over previous
"""GraphSAGE mean-aggregation layer on 8 Trainium2 NeuronCores (Bass/Bacc).

Math: out = D^{-1} A (x @ W + b)  ==  (D^{-1} A x) @ W + mask (outer) b
where A is the (row=dest, col=src) adjacency from edge_index, D = row degrees,
mask[d] = 1 if deg[d] > 0 else 0 (zero-degree rows are exactly 0 in the ref).

Strategy (one SPMD program on 8 cores, dest nodes sharded, bf16 data path):
  - Host: sort edges by dest, bucket into 128-dest windows (wpc per core).
    Within a window, edges are grouped by source QUADRANT (4 tables of
    25000 rows each so indices fit int16 for dma_gather), each group padded
    to the global per-quadrant tile count T_q with (idx=0, weight=0) slots.
    Per-edge weight 1/deg[dest] is folded into the one-hot S matrix.
  - Device, per window: 8 dma_gather calls (per quadrant: one 8-tile call +
    one (T_q-8)-tile call, capped at 1024 rows each by the SWDGE ring),
    round-robined over 4 SWDGE queues whose descriptor generation runs in
    parallel. Then per 128-edge tile a DVE-built weighted one-hot S (bf16)
    and a PE matmul S^T @ G accumulating into PSUM [128 dests, 256] fp32;
    transpose (bf16) + W matmul (bf16) + masked bias, DMA 128 rows out.
  - Raw bass engine programs with explicit semaphores; one sync wait per
    instruction (standalone wait_ge).
"""

import numpy as np
import ml_dtypes

import concourse.bass as bass
import concourse.bacc as bacc
import concourse.mybir as mybir
from concourse.bass_utils import run_bass_kernel_spmd
from concourse.library_config import mlp

P = 128
F = 256

N_NODES = 100000
N_CORES = 8
NPC = N_NODES // N_CORES  # dest rows per core
NQ = 4                    # source quadrant tables
QROWS = N_NODES // NQ     # rows per quadrant table (int16-addressable)
CAP_TILES = 8             # SWDGE ring: <=1024 descriptors per dma_gather call

BF16 = mybir.dt.bfloat16


def build_nc(npc, t_q):
    """One SPMD Bass program; t_q = tiles per (window, quadrant)."""
    wpc = (npc + P - 1) // P
    T = NQ * t_q  # edge tiles per 128-dest window
    f = F
    kf = f // P
    dt_f32 = mybir.dt.float32

    # per-quadrant gather calls: chunks of <= CAP_TILES tiles
    chunks = []
    t0 = 0
    while t0 < t_q:
        chunks.append((t0, min(CAP_TILES, t_q - t0)))
        t0 += CAP_TILES
    cols_per_q = t_q * P // 16     # idx table columns per (window, quadrant)
    cols_per_w = NQ * cols_per_q

    nc = bacc.Bacc("TRN2", num_swdge_queues=4)

    xq_h = [
        nc.declare_dram_parameter(f"x{q}", [QROWS, f], BF16, isOutput=False)
        for q in range(NQ)
    ]
    idx_h = nc.declare_dram_parameter(
        "srcidx", [P, wpc * cols_per_w], mybir.dt.int16, isOutput=False
    )
    sdata_h = nc.declare_dram_parameter(
        "sdata", [wpc * P, T * P], BF16, isOutput=False
    )
    msk_h = nc.declare_dram_parameter("maskw", [wpc, P], BF16, isOutput=False)
    w_h = nc.declare_dram_parameter("Wm", [f, f], BF16, isOutput=False)
    b_h = nc.declare_dram_parameter("bv", [1, f], BF16, isOutput=False)
    out_h = nc.declare_dram_parameter("out", [npc, f], dt_f32, isOutput=True)


    from contextlib import ExitStack

    ctx = ExitStack()
    with ctx:
        sb = lambda name, shape, dt: ctx.enter_context(nc.sbuf_tensor(name, shape, dt))
        ps = lambda name, shape, dt: ctx.enter_context(nc.psum_tensor(name, shape, dt))
        sem = lambda name: ctx.enter_context(nc.semaphore(name))

        iota_f = sb("iota_f", [P, P], BF16)
        ident = sb("ident", [P, P], BF16)
        w0 = sb("w0", [P, f], BF16)
        w1 = sb("w1", [P, f], BF16)
        b_sb = sb("b_sb", [1, f], BF16)
        idx_all = sb("idx_all", [P, wpc * cols_per_w], mybir.dt.int16)
        msk_t = sb("msk_t", [1, 2 * P], BF16)
        g_buf = sb("g_buf", [P, 2, T, f], BF16)
        s_buf = sb("s_buf", [P, 2, T, P], BF16)
        agg_sb = sb("agg_sb", [P, 2 * f], BF16)
        tp_sb = sb("tp_sb", [P, kf * P], BF16)
        out_sb = sb("out_sb", [P, 2 * f], dt_f32)
        agg_ps = [ps("agg_ps0", [P, f], dt_f32), ps("agg_ps1", [P, f], dt_f32)]
        tp_ps = [ps("tp_ps0", [P, P], BF16), ps("tp_ps1", [P, P], BF16)]
        out_ps = [ps("out_ps0", [P, f], dt_f32), ps("out_ps1", [P, f], dt_f32)]
        SEM_META = sem("sem_meta")
        SEM_CONST = sem("sem_const")
        SEM_GW = [[sem(f"sem_gw{par}q{q}") for q in range(NQ)] for par in range(2)]
        SEM_SLD = [sem("sem_sld0"), sem("sem_sld1")]
        SEM_MM = sem("sem_mm")
        SEM_CP = sem("sem_cp")
        SEM_TP = sem("sem_tp")
        SEM_TPC = sem("sem_tpc")
        SEM_FIN = sem("sem_fin")
        SEM_OUT = sem("sem_out")
        SEM_OD = sem("sem_od")
        SEM_MSK = [sem("sem_msk0"), sem("sem_msk1")]

        w_sb = [w0, w1]
        calls_per_w = NQ * len(chunks)

        with nc.Block() as block:

            @block.sync
            def _(sync):
                # startup loads (HWDGE)
                sync.dma_start(w0[:, :], w_h[0:P, :]).then_inc(SEM_META, 16)
                sync.dma_start(w1[:, :], w_h[P : 2 * P, :]).then_inc(SEM_META, 16)
                sync.dma_start(b_sb[:, :], b_h[:, :]).then_inc(SEM_META, 16)
                sync.dma_start(idx_all[:, :], idx_h[:, :]).then_inc(SEM_META, 16)
                # per-window: prefetch S + mask, store previous window's output
                for W in range(wpc + 1):
                    if W < wpc:
                        if W >= 2:
                            sync.wait_ge(SEM_MM, (W - 1) * T)  # s_buf parity free
                        sync.dma_start(
                            s_buf[:, W % 2, :, :], sdata_h[W * P : (W + 1) * P, :]
                        ).then_inc(SEM_SLD[W % 2], 16)
                        if W >= 2:
                            sync.wait_ge(SEM_FIN, W - 1)  # msk_t slot free
                        sync.dma_start(
                            msk_t[:1, (W % 2) * P : (W % 2) * P + P], msk_h[W : W + 1, :]
                        ).then_inc(SEM_MSK[W % 2], 16)
                    if W >= 1:
                        V = W - 1
                        rows = min(P, npc - V * P)
                        ob = (V % 2) * f
                        sync.wait_ge(SEM_OUT, V + 1)
                        sync.dma_start(
                            out_h[V * P : V * P + rows, :], out_sb[:rows, ob : ob + f]
                        ).then_inc(SEM_OD, 16)

            @block.gpsimd
            def _(gpsimd):
                gpsimd.load_library(mlp)
                # constants (gpsimd ops are unordered across DSP cores: sync each)
                gpsimd.iota(
                    iota_f[:, :],
                    pattern=[[1, P]],
                    base=0,
                    channel_multiplier=0,
                    allow_small_or_imprecise_dtypes=True,
                ).then_inc(SEM_CONST, 1)
                gpsimd.memset(ident[:, :], 0.0).then_inc(SEM_CONST, 1)
                gpsimd.wait_ge(SEM_CONST, 2)
                gpsimd.affine_select(
                    out=ident[:, :],
                    in_=ident[:, :],
                    compare_op=mybir.AluOpType.not_equal,
                    fill=1.0,
                    base=0,
                    pattern=[[-1, P]],
                    channel_multiplier=1,
                ).then_inc(SEM_CONST, 1)
                nregs = {nt: gpsimd.to_reg(nt * P) for _, nt in set(chunks)}
                # gathers
                gpsimd.wait_ge(SEM_META, 64)
                for W in range(wpc):
                    par = W % 2
                    if W >= 2:
                        # g_buf parity slot free once window W-2 is consumed
                        gpsimd.wait_ge(SEM_MM, (W - 1) * T)
                    for q in range(NQ):
                        cbase = W * cols_per_w + q * cols_per_q
                        for c0, nt in chunks:
                            gpsimd.dma_gather(
                                g_buf[:, par, q * t_q + c0 : q * t_q + c0 + nt, :],
                                xq_h[q][:, :],
                                idx_all[
                                    :,
                                    cbase + c0 * 8 : cbase + (c0 + nt) * 8,
                                ],
                                nt * P,
                                nregs[nt],
                                f,
                                queue_num=q,
                            ).then_inc(SEM_GW[par][q], 16)

            @block.vector
            def _(vector):
                vector.wait_ge(SEM_CONST, 3)
                vector.wait_ge(SEM_META, 64)
                for W in range(wpc):
                    # copy window aggregate out of PSUM (f32 -> bf16)
                    ab = (W % 2) * f
                    vector.wait_ge(SEM_MM, (W + 1) * T)
                    vector.tensor_copy(
                        agg_sb[:, ab : ab + f], agg_ps[W % 2][:, :]
                    ).then_inc(SEM_CP, 1)
                    # copy transposes out of PSUM
                    for k in range(kf):
                        vector.wait_ge(SEM_TP, kf * W + k + 1)
                        vector.tensor_copy(
                            tp_sb[:, k * P : (k + 1) * P], tp_ps[k][:, :]
                        ).then_inc(SEM_TPC, 1)
                    # copy final output out of PSUM
                    ob = (W % 2) * f
                    if W >= 2:
                        vector.wait_ge(SEM_OD, (W - 1) * 16)
                    vector.wait_ge(SEM_FIN, W + 1)
                    vector.tensor_copy(
                        out_sb[:, ob : ob + f], out_ps[W % 2][:, :]
                    ).then_inc(SEM_OUT, 1)

            @block.tensor
            def _(tensor):
                tensor.wait_ge(SEM_META, 64)
                tensor.wait_ge(SEM_CONST, 3)
                for W in range(wpc):
                    par = W % 2
                    ab = par * f
                    if W >= 2:
                        tensor.wait_ge(SEM_CP, W - 1)  # agg bank free
                    tensor.wait_ge(SEM_SLD[par], 16 * (W // 2 + 1))  # window S loaded
                    for t in range(T):
                        i = W * T + t
                        if t % t_q == 0:  # quadrant gathered (per queue)
                            tensor.wait_ge(
                                SEM_GW[par][t // t_q],
                                16 * len(chunks) * (W // 2 + 1),
                            )
                        tensor.matmul(
                            agg_ps[par][:, :],
                            s_buf[:, par, t, :],
                            g_buf[:, par, t, :],
                            start=(t == 0),
                            stop=(t == T - 1),
                        ).then_inc(SEM_MM, 1)
                    tensor.wait_ge(SEM_CP, W + 1)  # agg_sb ready
                    for k in range(kf):
                        if W >= 1:
                            tensor.wait_ge(SEM_TPC, kf * (W - 1) + k + 1)  # tp bank free
                        tensor.transpose(
                            tp_ps[k][:, :],
                            agg_sb[:, ab + k * P : ab + (k + 1) * P],
                            ident[:, :],
                        ).then_inc(SEM_TP, 1)
                    ob = par * f
                    if W >= 2:
                        tensor.wait_ge(SEM_OUT, W - 1)  # out_ps bank free
                    for k in range(kf):
                        tensor.wait_ge(SEM_TPC, kf * W + k + 1)  # tp_sb ready
                        tensor.matmul(
                            out_ps[par][:, :],
                            tp_sb[:, k * P : (k + 1) * P],
                            w_sb[k][:, :],
                            start=(k == 0),
                            stop=False,
                        )
                    tensor.wait_ge(SEM_MSK[par], 16 * (W // 2 + 1))
                    tensor.matmul(
                        out_ps[par][:, :],
                        msk_t[:1, par * P : par * P + P],
                        b_sb[:1, :],
                        start=False,
                        stop=True,
                    ).then_inc(SEM_FIN, 1)

    nc.compile()
    return nc


def prepare_inputs(x, edge_index, W, b, n_cores=N_CORES):
    """Host-side: sort/bucket edges by destination into per-core padded windows,
    grouped by source quadrant."""
    n = x.shape[0]
    npc = n // n_cores
    wpc = (npc + P - 1) // P

    row = np.asarray(edge_index[0], dtype=np.int64)  # dest
    col = np.asarray(edge_index[1], dtype=np.int64)  # src

    deg = np.bincount(row, minlength=n).astype(np.float64)
    invdeg = np.zeros(n, dtype=np.float64)
    nz = deg > 0
    invdeg[nz] = 1.0 / deg[nz]

    # group edges by (core, window, src-quadrant); order within a run is free
    # (each edge carries its own dest label into the one-hot S matrix)
    quad = col // QROWS
    core0 = row // npc
    win0 = (row - core0 * npc) // P
    order = np.lexsort((quad, core0 * wpc + win0))
    row_s = row[order]
    col_s = col[order]
    quad_s = quad[order]

    core_of = row_s // npc
    local = row_s - core_of * npc
    win = local // P
    gslot = ((core_of * wpc) + win) * NQ + quad_s  # global (window, quadrant) run

    n_runs = n_cores * wpc * NQ
    counts = np.bincount(gslot, minlength=n_runs)
    t_q = max(1, int(np.ceil(counts.max() / P)))

    first = np.searchsorted(gslot, np.arange(n_runs))
    pos = np.arange(len(gslot)) - first[gslot]  # position within the run

    T = NQ * t_q
    rows_q = t_q * P

    # per (core, window, quadrant): padded run of rows_q edge slots
    # srcidx layout: [core][window][quadrant][wrapped16: (col, p16)]
    # pad slots carry weight 0 but still gather a row: spread them across the
    # table so duplicate-address reads don't serialize on one HBM line
    pad_rows = ((np.arange(rows_q, dtype=np.int64) * 97) % QROWS).astype(np.int16)
    srcidx = np.broadcast_to(
        pad_rows, (n_cores, wpc, NQ, rows_q)
    ).copy()
    src_local = (col_s - quad_s * QROWS).astype(np.int16)
    srcidx[core_of, win, quad_s, pos] = src_local
    tile = quad_s * t_q + pos // P

    # host-built weighted one-hot S tiles: svals[c, w, t, e, d] = 1/deg
    svals = np.zeros((n_cores, wpc, T, P, P), dtype=ml_dtypes.bfloat16)
    svals[core_of, win, tile, pos % P, local % P] = invdeg[row_s].astype(
        ml_dtypes.bfloat16
    )

    # wrap-16: in-run row j -> (partition j%16, column j//16), tiled to 128
    srcidx = srcidx.reshape(n_cores, wpc, NQ, rows_q // 16, 16)
    srcidx = np.transpose(srcidx, (0, 4, 1, 2, 3))  # [cores, 16, wpc, NQ, cols]
    srcidx = np.tile(srcidx, (1, 8, 1, 1, 1)).reshape(n_cores, P, -1)

    # sdata layout: [wpc*P partitions-rows, T*P]: window-major, edge-partition
    # rows, (tile, dest) columns -- matches the s_buf[:, par, :, :] DMA dest
    sdata = np.ascontiguousarray(svals.transpose(0, 1, 3, 2, 4)).reshape(
        n_cores, wpc * P, T * P
    )
    del svals

    maskw = np.zeros((n_cores, wpc * P), dtype=np.float32)
    maskw[:, :npc] = nz.astype(np.float32).reshape(n_cores, npc)
    maskw = maskw.reshape(n_cores, wpc, P).astype(ml_dtypes.bfloat16)

    x_bf = np.ascontiguousarray(np.asarray(x, dtype=np.float32)).astype(
        ml_dtypes.bfloat16
    )
    xq = [np.ascontiguousarray(x_bf[q * QROWS : (q + 1) * QROWS]) for q in range(NQ)]
    W_bf = np.ascontiguousarray(np.asarray(W, dtype=np.float32)).astype(
        ml_dtypes.bfloat16
    )
    b_bf = (
        np.ascontiguousarray(np.asarray(b, dtype=np.float32))
        .astype(ml_dtypes.bfloat16)
        .reshape(1, -1)
    )

    per_core = []
    for c in range(n_cores):
        m = {f"x{q}": xq[q] for q in range(NQ)}
        m["srcidx"] = np.ascontiguousarray(srcidx[c])
        m["sdata"] = sdata[c]
        m["maskw"] = maskw[c]
        m["Wm"] = W_bf
        m["bv"] = b_bf
        per_core.append(m)
    return per_core, t_q


def run(x, edge_index, W, b, n_cores=N_CORES, trace=False):
    n, f = x.shape
    npc = n // n_cores
    in_maps, t_q = prepare_inputs(x, edge_index, W, b, n_cores)
    nc = build_nc(npc, t_q)
    res = run_bass_kernel_spmd(nc, in_maps, list(range(n_cores)), trace=trace)
    out = np.concatenate([res.results[c]["out"] for c in range(n_cores)], axis=0)
    return out, res


def kernel(x, edge_index, W, b):
    out, _ = run(np.asarray(x), np.asarray(edge_index), np.asarray(W), np.asarray(b))
    return out.astype(np.float32)


# revision 14
# speedup vs baseline: 3.8569x; 1.2466x over previous
"""GraphSAGE mean-aggregation layer on 8 Trainium2 NeuronCores (Bass/Bacc).

Math: out = D^{-1} A (x @ W + b)  ==  (D^{-1} A x) @ W + mask (outer) b
where A is the (row=dest, col=src) adjacency from edge_index, D = row degrees,
mask[d] = 1 if deg[d] > 0 else 0 (zero-degree rows are exactly 0 in the ref).

Strategy (one SPMD program on 8 cores, dest nodes sharded, bf16 data path):
  - Host: sort edges by dest, bucket into 128-dest windows (wpc per core).
    Within a window, edges are grouped by source QUADRANT (4 tables of
    25000 rows each so indices fit int16 for dma_gather), each group padded
    to the global per-quadrant tile count T_q with (idx=0, weight=0) slots.
    Per-edge weight 1/deg[dest] is folded into the one-hot S matrix.
  - Device, per window: 8 dma_gather calls (per quadrant: one 8-tile call +
    one (T_q-8)-tile call, capped at 1024 rows each by the SWDGE ring),
    round-robined over 4 SWDGE queues whose descriptor generation runs in
    parallel. Then per 128-edge tile a DVE-built weighted one-hot S (bf16)
    and a PE matmul S^T @ G accumulating into PSUM [128 dests, 256] fp32;
    transpose (bf16) + W matmul (bf16) + masked bias, DMA 128 rows out.
  - Raw bass engine programs with explicit semaphores; one sync wait per
    instruction (standalone wait_ge).
"""

import numpy as np
import ml_dtypes

import concourse.bass as bass
import concourse.bacc as bacc
import concourse.mybir as mybir
from concourse.bass_utils import run_bass_kernel_spmd
from concourse.library_config import mlp

P = 128
F = 256

N_NODES = 100000
N_CORES = 8
NPC = N_NODES // N_CORES  # dest rows per core
NQ = 4                    # source quadrant tables
QROWS = N_NODES // NQ     # rows per quadrant table (int16-addressable)
CAP_TILES = 8             # SWDGE ring: <=1024 descriptors per dma_gather call

BF16 = mybir.dt.bfloat16


def build_nc(npc, t_q):
    """One SPMD Bass program; t_q = tiles per (window, quadrant)."""
    wpc = (npc + P - 1) // P
    T = NQ * t_q  # edge tiles per 128-dest window
    f = F
    kf = f // P
    dt_f32 = mybir.dt.float32

    # per-quadrant gather calls: chunks of <= CAP_TILES tiles
    chunks = []
    t0 = 0
    while t0 < t_q:
        chunks.append((t0, min(CAP_TILES, t_q - t0)))
        t0 += CAP_TILES
    cols_per_q = t_q * P // 16     # idx table columns per (window, quadrant)
    cols_per_w = NQ * cols_per_q

    nc = bacc.Bacc("TRN2", num_swdge_queues=4)

    xq_h = [
        nc.declare_dram_parameter(f"x{q}", [QROWS, f], BF16, isOutput=False)
        for q in range(NQ)
    ]
    idx_h = nc.declare_dram_parameter(
        "srcidx", [P, wpc * cols_per_w], mybir.dt.int16, isOutput=False
    )
    sdata_h = nc.declare_dram_parameter(
        "sdata", [wpc * P, T * P], BF16, isOutput=False
    )
    msk_h = nc.declare_dram_parameter("maskw", [wpc, P], BF16, isOutput=False)
    w_h = nc.declare_dram_parameter("Wm", [f, f], BF16, isOutput=False)
    b_h = nc.declare_dram_parameter("bv", [1, f], BF16, isOutput=False)
    out_h = nc.declare_dram_parameter("out", [npc, f], dt_f32, isOutput=True)


    from contextlib import ExitStack

    ctx = ExitStack()
    with ctx:
        sb = lambda name, shape, dt: ctx.enter_context(nc.sbuf_tensor(name, shape, dt))
        ps = lambda name, shape, dt: ctx.enter_context(nc.psum_tensor(name, shape, dt))
        sem = lambda name: ctx.enter_context(nc.semaphore(name))

        iota_f = sb("iota_f", [P, P], BF16)
        ident = sb("ident", [P, P], BF16)
        w0 = sb("w0", [P, f], BF16)
        w1 = sb("w1", [P, f], BF16)
        b_sb = sb("b_sb", [1, f], BF16)
        idx_all = sb("idx_all", [P, wpc * cols_per_w], mybir.dt.int16)
        msk_t = sb("msk_t", [1, 2 * P], BF16)
        g_buf = sb("g_buf", [P, 4, T, f], BF16)
        s_buf = sb("s_buf", [P, 2, T, P], BF16)
        agg_sb = sb("agg_sb", [P, 2 * f], BF16)
        tp_sb = sb("tp_sb", [P, kf * P], BF16)
        out_sb = sb("out_sb", [P, 2 * f], dt_f32)
        agg_ps = [ps("agg_ps0", [P, f], dt_f32), ps("agg_ps1", [P, f], dt_f32)]
        tp_ps = [ps("tp_ps0", [P, P], BF16), ps("tp_ps1", [P, P], BF16)]
        out_ps = [ps("out_ps0", [P, f], dt_f32), ps("out_ps1", [P, f], dt_f32)]
        SEM_META = sem("sem_meta")
        SEM_CONST = sem("sem_const")
        SEM_GW = [[sem(f"sem_gw{par}q{q}") for q in range(NQ)] for par in range(4)]
        SEM_SLD = [sem("sem_sld0"), sem("sem_sld1")]
        SEM_MM = sem("sem_mm")
        SEM_CP = sem("sem_cp")
        SEM_TP = sem("sem_tp")
        SEM_TPC = sem("sem_tpc")
        SEM_FIN = sem("sem_fin")
        SEM_OUT = sem("sem_out")
        SEM_OD = sem("sem_od")
        SEM_MSK = [sem("sem_msk0"), sem("sem_msk1")]

        w_sb = [w0, w1]
        calls_per_w = NQ * len(chunks)

        with nc.Block() as block:

            @block.sync
            def _(sync):
                # startup loads (HWDGE)
                sync.dma_start(w0[:, :], w_h[0:P, :]).then_inc(SEM_META, 16)
                sync.dma_start(w1[:, :], w_h[P : 2 * P, :]).then_inc(SEM_META, 16)
                sync.dma_start(b_sb[:, :], b_h[:, :]).then_inc(SEM_META, 16)
                sync.dma_start(idx_all[:, :], idx_h[:, :]).then_inc(SEM_META, 16)
                # per-window: prefetch S + mask, store previous window's output
                for W in range(wpc + 1):
                    if W < wpc:
                        if W >= 2:
                            sync.wait_ge(SEM_MM, (W - 1) * T)  # s_buf parity free
                        sync.dma_start(
                            s_buf[:, W % 2, :, :], sdata_h[W * P : (W + 1) * P, :]
                        ).then_inc(SEM_SLD[W % 2], 16)
                        if W >= 2:
                            sync.wait_ge(SEM_FIN, W - 1)  # msk_t slot free
                        sync.dma_start(
                            msk_t[:1, (W % 2) * P : (W % 2) * P + P], msk_h[W : W + 1, :]
                        ).then_inc(SEM_MSK[W % 2], 16)
                    if W >= 1:
                        V = W - 1
                        rows = min(P, npc - V * P)
                        ob = (V % 2) * f
                        sync.wait_ge(SEM_OUT, V + 1)
                        sync.dma_start(
                            out_h[V * P : V * P + rows, :], out_sb[:rows, ob : ob + f]
                        ).then_inc(SEM_OD, 16)

            @block.gpsimd
            def _(gpsimd):
                gpsimd.load_library(mlp)
                # constants (gpsimd ops are unordered across DSP cores: sync each)
                gpsimd.iota(
                    iota_f[:, :],
                    pattern=[[1, P]],
                    base=0,
                    channel_multiplier=0,
                    allow_small_or_imprecise_dtypes=True,
                ).then_inc(SEM_CONST, 1)
                gpsimd.memset(ident[:, :], 0.0).then_inc(SEM_CONST, 1)
                gpsimd.wait_ge(SEM_CONST, 2)
                gpsimd.affine_select(
                    out=ident[:, :],
                    in_=ident[:, :],
                    compare_op=mybir.AluOpType.not_equal,
                    fill=1.0,
                    base=0,
                    pattern=[[-1, P]],
                    channel_multiplier=1,
                ).then_inc(SEM_CONST, 1)
                nregs = {nt: gpsimd.to_reg(nt * P) for _, nt in set(chunks)}
                # gathers
                gpsimd.wait_ge(SEM_META, 64)
                for W in range(wpc):
                    slot = W % 4
                    if W >= 4:
                        # g_buf slot free once window W-4 is consumed
                        gpsimd.wait_ge(SEM_MM, (W - 3) * T)
                    for q in range(NQ):
                        cbase = W * cols_per_w + q * cols_per_q
                        for c0, nt in chunks:
                            gpsimd.dma_gather(
                                g_buf[:, slot, q * t_q + c0 : q * t_q + c0 + nt, :],
                                xq_h[q][:, :],
                                idx_all[
                                    :,
                                    cbase + c0 * 8 : cbase + (c0 + nt) * 8,
                                ],
                                nt * P,
                                nregs[nt],
                                f,
                                queue_num=q,
                            ).then_inc(SEM_GW[slot][q], 16)

            @block.vector
            def _(vector):
                vector.wait_ge(SEM_CONST, 3)
                vector.wait_ge(SEM_META, 64)
                for W in range(wpc):
                    # copy window aggregate out of PSUM (f32 -> bf16)
                    ab = (W % 2) * f
                    vector.wait_ge(SEM_MM, (W + 1) * T)
                    vector.tensor_copy(
                        agg_sb[:, ab : ab + f], agg_ps[W % 2][:, :]
                    ).then_inc(SEM_CP, 1)
                    # copy transposes out of PSUM
                    for k in range(kf):
                        vector.wait_ge(SEM_TP, kf * W + k + 1)
                        vector.tensor_copy(
                            tp_sb[:, k * P : (k + 1) * P], tp_ps[k][:, :]
                        ).then_inc(SEM_TPC, 1)
                    # copy final output out of PSUM
                    ob = (W % 2) * f
                    if W >= 2:
                        vector.wait_ge(SEM_OD, (W - 1) * 16)
                    vector.wait_ge(SEM_FIN, W + 1)
                    vector.tensor_copy(
                        out_sb[:, ob : ob + f], out_ps[W % 2][:, :]
                    ).then_inc(SEM_OUT, 1)

            @block.tensor
            def _(tensor):
                tensor.wait_ge(SEM_META, 64)
                tensor.wait_ge(SEM_CONST, 3)
                for W in range(wpc):
                    par = W % 2
                    ab = par * f
                    if W >= 2:
                        tensor.wait_ge(SEM_CP, W - 1)  # agg bank free
                    tensor.wait_ge(SEM_SLD[par], 16 * (W // 2 + 1))  # window S loaded
                    for t in range(T):
                        i = W * T + t
                        if t % t_q == 0:  # quadrant gathered (per queue)
                            tensor.wait_ge(
                                SEM_GW[W % 4][t // t_q],
                                16 * len(chunks) * (W // 4 + 1),
                            )
                        tensor.matmul(
                            agg_ps[par][:, :],
                            s_buf[:, par, t, :],
                            g_buf[:, W % 4, t, :],
                            start=(t == 0),
                            stop=(t == T - 1),
                        ).then_inc(SEM_MM, 1)
                    tensor.wait_ge(SEM_CP, W + 1)  # agg_sb ready
                    for k in range(kf):
                        if W >= 1:
                            tensor.wait_ge(SEM_TPC, kf * (W - 1) + k + 1)  # tp bank free
                        tensor.transpose(
                            tp_ps[k][:, :],
                            agg_sb[:, ab + k * P : ab + (k + 1) * P],
                            ident[:, :],
                        ).then_inc(SEM_TP, 1)
                    ob = par * f
                    if W >= 2:
                        tensor.wait_ge(SEM_OUT, W - 1)  # out_ps bank free
                    for k in range(kf):
                        tensor.wait_ge(SEM_TPC, kf * W + k + 1)  # tp_sb ready
                        tensor.matmul(
                            out_ps[par][:, :],
                            tp_sb[:, k * P : (k + 1) * P],
                            w_sb[k][:, :],
                            start=(k == 0),
                            stop=False,
                        )
                    tensor.wait_ge(SEM_MSK[par], 16 * (W // 2 + 1))
                    tensor.matmul(
                        out_ps[par][:, :],
                        msk_t[:1, par * P : par * P + P],
                        b_sb[:1, :],
                        start=False,
                        stop=True,
                    ).then_inc(SEM_FIN, 1)

    nc.compile()
    return nc


def prepare_inputs(x, edge_index, W, b, n_cores=N_CORES):
    """Host-side: sort/bucket edges by destination into per-core padded windows,
    grouped by source quadrant."""
    n = x.shape[0]
    npc = n // n_cores
    wpc = (npc + P - 1) // P

    row = np.asarray(edge_index[0], dtype=np.int64)  # dest
    col = np.asarray(edge_index[1], dtype=np.int64)  # src

    deg = np.bincount(row, minlength=n).astype(np.float64)
    invdeg = np.zeros(n, dtype=np.float64)
    nz = deg > 0
    invdeg[nz] = 1.0 / deg[nz]

    # group edges by (core, window, src-quadrant); order within a run is free
    # (each edge carries its own dest label into the one-hot S matrix)
    quad = col // QROWS
    core0 = row // npc
    win0 = (row - core0 * npc) // P
    order = np.lexsort((quad, core0 * wpc + win0))
    row_s = row[order]
    col_s = col[order]
    quad_s = quad[order]

    core_of = row_s // npc
    local = row_s - core_of * npc
    win = local // P
    gslot = ((core_of * wpc) + win) * NQ + quad_s  # global (window, quadrant) run

    n_runs = n_cores * wpc * NQ
    counts = np.bincount(gslot, minlength=n_runs)
    t_q = max(1, int(np.ceil(counts.max() / P)))

    first = np.searchsorted(gslot, np.arange(n_runs))
    pos = np.arange(len(gslot)) - first[gslot]  # position within the run

    T = NQ * t_q
    rows_q = t_q * P

    # per (core, window, quadrant): padded run of rows_q edge slots
    # srcidx layout: [core][window][quadrant][wrapped16: (col, p16)]
    # pad slots carry weight 0 but still gather a row: spread them across the
    # table so duplicate-address reads don't serialize on one HBM line
    pad_rows = ((np.arange(rows_q, dtype=np.int64) * 97) % QROWS).astype(np.int16)
    srcidx = np.broadcast_to(
        pad_rows, (n_cores, wpc, NQ, rows_q)
    ).copy()
    src_local = (col_s - quad_s * QROWS).astype(np.int16)
    srcidx[core_of, win, quad_s, pos] = src_local
    tile = quad_s * t_q + pos // P

    # host-built weighted one-hot S tiles: svals[c, w, t, e, d] = 1/deg
    svals = np.zeros((n_cores, wpc, T, P, P), dtype=ml_dtypes.bfloat16)
    svals[core_of, win, tile, pos % P, local % P] = invdeg[row_s].astype(
        ml_dtypes.bfloat16
    )

    # wrap-16: in-run row j -> (partition j%16, column j//16), tiled to 128
    srcidx = srcidx.reshape(n_cores, wpc, NQ, rows_q // 16, 16)
    srcidx = np.transpose(srcidx, (0, 4, 1, 2, 3))  # [cores, 16, wpc, NQ, cols]
    srcidx = np.tile(srcidx, (1, 8, 1, 1, 1)).reshape(n_cores, P, -1)

    # sdata layout: [wpc*P partitions-rows, T*P]: window-major, edge-partition
    # rows, (tile, dest) columns -- matches the s_buf[:, par, :, :] DMA dest
    sdata = np.ascontiguousarray(svals.transpose(0, 1, 3, 2, 4)).reshape(
        n_cores, wpc * P, T * P
    )
    del svals

    maskw = np.zeros((n_cores, wpc * P), dtype=np.float32)
    maskw[:, :npc] = nz.astype(np.float32).reshape(n_cores, npc)
    maskw = maskw.reshape(n_cores, wpc, P).astype(ml_dtypes.bfloat16)

    x_bf = np.ascontiguousarray(np.asarray(x, dtype=np.float32)).astype(
        ml_dtypes.bfloat16
    )
    xq = [np.ascontiguousarray(x_bf[q * QROWS : (q + 1) * QROWS]) for q in range(NQ)]
    W_bf = np.ascontiguousarray(np.asarray(W, dtype=np.float32)).astype(
        ml_dtypes.bfloat16
    )
    b_bf = (
        np.ascontiguousarray(np.asarray(b, dtype=np.float32))
        .astype(ml_dtypes.bfloat16)
        .reshape(1, -1)
    )

    per_core = []
    for c in range(n_cores):
        m = {f"x{q}": xq[q] for q in range(NQ)}
        m["srcidx"] = np.ascontiguousarray(srcidx[c])
        m["sdata"] = sdata[c]
        m["maskw"] = maskw[c]
        m["Wm"] = W_bf
        m["bv"] = b_bf
        per_core.append(m)
    return per_core, t_q


def run(x, edge_index, W, b, n_cores=N_CORES, trace=False):
    n, f = x.shape
    npc = n // n_cores
    in_maps, t_q = prepare_inputs(x, edge_index, W, b, n_cores)
    nc = build_nc(npc, t_q)
    res = run_bass_kernel_spmd(nc, in_maps, list(range(n_cores)), trace=trace)
    out = np.concatenate([res.results[c]["out"] for c in range(n_cores)], axis=0)
    return out, res


def kernel(x, edge_index, W, b):
    out, _ = run(np.asarray(x), np.asarray(edge_index), np.asarray(W), np.asarray(b))
    return out.astype(np.float32)
